# revision 1
# baseline (speedup 1.0000x reference)
"""Trainium2 Bass kernel for the Clements photonic mesh (N=512, L=512).

Column-sharded across 8 NeuronCores: every PC/MMI layer is a row operation
(left multiply), so each core evolves its own 64-column block of the
512x512 complex transfer matrix with zero communication.

Device layout (per core):
  Rows are split by parity into E (even rows 0,2,..,510) and O (odd rows),
  each linearized to 256 entries stored as two 128-partition tiles (t=0/1).
  State tiles are [128, 2(X/Y), 2(t), 64cols] fp32 (X=Re, Y=Im).
  Per-pair layer coefficients live at one partition per pair, so every
  elementwise op is a tensor_scalar / scalar_tensor_tensor with a [128,1]
  per-partition scalar slice.
  Even layers pair E[k] with O[k] (partition aligned).  Odd layers pair
  O[k] with E[k+1]; the +1 partition shift is done on the TensorEngine with
  constant shift matrices (engines cannot read cross-partition).
"""

import sys

sys.path.insert(0, "/opt/trn_rl_repo")

from contextlib import ExitStack

import numpy as np

import concourse.bass as bass
import concourse.tile as tile
from concourse import bacc, mybir
from concourse.bass_utils import run_bass_kernel_spmd

F32 = mybir.dt.float32
ALU = mybir.AluOpType
ACTF = mybir.ActivationFunctionType

N = 512
L = 512
NPAIR_E = 256
NPAIR_O = 255
TWO_PI = 6.283185307179586
HALF_PI = 1.5707963267948966
NCORES = 8
CPD = N // NCORES  # columns per device = 64

_CACHE = {}


def _build_program(n_steps=L // 2):
    """One scan step = 2 even layers + 2 odd layers (matches reference)."""
    nc = bacc.Bacc("TRN2", target_bir_lowering=False)

    par = {}
    for nm in (
        "the0", "the1", "le0", "le1", "ie0", "ie1",
        "tho0", "tho1", "lo0", "lo1", "io0", "io1",
    ):
        par[nm] = nc.declare_dram_parameter(nm, [128, L], F32, isOutput=False)
    par["pout"] = nc.declare_dram_parameter("pout", [128, 4], F32, isOutput=False)
    par["init_e"] = nc.declare_dram_parameter("init_e", [128, 2, CPD], F32, isOutput=False)
    par["init_o"] = nc.declare_dram_parameter("init_o", [128, 2, CPD], F32, isOutput=False)
    par["shifts"] = nc.declare_dram_parameter("shifts", [128, 5, 128], F32, isOutput=False)
    outv = nc.declare_dram_parameter("outv", [128, 2, 2, 2, CPD], F32, isOutput=True)

    with tile.TileContext(nc) as tc, ExitStack() as ctx:
        consts = ctx.enter_context(tc.tile_pool(name="consts", bufs=1))
        coefp = ctx.enter_context(tc.tile_pool(name="coefs", bufs=1))
        srcp = ctx.enter_context(tc.tile_pool(name="srcs", bufs=1))
        stp = ctx.enter_context(tc.tile_pool(name="state", bufs=1))
        stgp = ctx.enter_context(tc.tile_pool(name="stage", bufs=2))
        psp = ctx.enter_context(tc.tile_pool(name="psum", bufs=2, space="PSUM"))

        shifts = consts.tile([128, 5, 128], F32, tag="shifts")
        nc.sync.dma_start(out=shifts[:], in_=par["shifts"][:])
        nhalfpi = consts.tile([128, 1], F32, tag="nhalfpi")
        nc.vector.memset(nhalfpi[:], -HALF_PI)

        SINP = (-0.16666666639369604, 0.0083333316715976, -0.00019840942043806986,
                2.752917460996653e-06, -2.3955613511594512e-08)
        COSP = (-0.49999999647064386, 0.041666645176626854, -0.0013888464831511677,
                2.4765157753536994e-05, -2.6136488530828197e-07)
        PI_HI = 3.1415927410125732
        PI_LO = -8.742278012618954e-08

        def cos_sin(c_out, s_out, th, tagp):
            """cos/sin of th in [0, 2pi] to ~1ulp f32, bias ~6e-10 rad.

            Quadrant reduction with double-f32 pi: both reduction branches are
            Sterbenz-exact in f32, the residual lo-part is a tiny constant
            selected by (sign, fold) masks and applied as a first-order
            rotation.  A plain fp32 reduction has ~2.6e-8 rad systematic bias
            which compounds coherently over ~1024 phase layers.
            """
            shape = list(th.shape)
            t_ = lambda nm: srcp.tile(shape, F32, tag=f"{tagp}{nm}", name=f"{tagp}{nm}")
            z, nz, a, mm, m2, p, acc, msk, w, sm = (
                t_(n) for n in ("z", "nz", "a", "mm", "m2", "p", "acc", "msk", "w", "sm"))
            v = nc.vector
            v.tensor_scalar(out=z[:], in0=th[:], scalar1=-PI_HI, scalar2=None, op0=ALU.add)
            v.tensor_scalar(out=nz[:], in0=z[:], scalar1=-1.0, scalar2=None, op0=ALU.mult)
            v.tensor_tensor(out=a[:], in0=nz[:], in1=z[:], op=ALU.max)          # |z|
            v.tensor_scalar(out=mm[:], in0=a[:], scalar1=-1.0, scalar2=PI_HI,
                            op0=ALU.mult, op1=ALU.add)                          # pi_hi-|z|
            v.tensor_scalar(out=msk[:], in0=mm[:], scalar1=1.0, scalar2=None,
                            op0=ALU.bypass)                                     # copy fold arm
            v.tensor_tensor(out=mm[:], in0=mm[:], in1=a[:], op=ALU.min)         # folded angle
            v.tensor_tensor(out=msk[:], in0=a[:], in1=msk[:], op=ALU.is_gt)     # a > pi_hi-a
            # mm_lo = PI_LO * (msk + sign(z)*(2*msk - 1))
            nc.scalar.sign(w[:], z[:])
            v.tensor_scalar(out=acc[:], in0=msk[:], scalar1=2.0, scalar2=-1.0,
                            op0=ALU.mult, op1=ALU.add)
            v.tensor_mul(w[:], w[:], acc[:])
            v.tensor_add(w[:], w[:], msk[:])
            v.tensor_scalar(out=w[:], in0=w[:], scalar1=PI_LO, scalar2=None, op0=ALU.mult)
            v.tensor_mul(m2[:], mm[:], mm[:])
            # sin(mm) = mm + mm^3 * P(m2)
            v.tensor_scalar(out=p[:], in0=m2[:], scalar1=SINP[4], scalar2=SINP[3],
                            op0=ALU.mult, op1=ALU.add)
            for cf in (SINP[2], SINP[1], SINP[0]):
                v.tensor_mul(p[:], p[:], m2[:])
                v.tensor_scalar(out=p[:], in0=p[:], scalar1=cf, scalar2=None, op0=ALU.add)
            v.tensor_mul(acc[:], m2[:], mm[:])
            v.tensor_mul(p[:], p[:], acc[:])
            v.tensor_add(sm[:], p[:], mm[:])                                    # sin(mm)
            # cos(mm) = 1 + m2 * Q(m2)
            v.tensor_scalar(out=p[:], in0=m2[:], scalar1=COSP[4], scalar2=COSP[3],
                            op0=ALU.mult, op1=ALU.add)
            for cf in (COSP[2], COSP[1], COSP[0]):
                v.tensor_mul(p[:], p[:], m2[:])
                v.tensor_scalar(out=p[:], in0=p[:], scalar1=cf, scalar2=None, op0=ALU.add)
            v.tensor_mul(p[:], p[:], m2[:])
            v.tensor_scalar(out=p[:], in0=p[:], scalar1=1.0, scalar2=None, op0=ALU.add)
            # first-order rotation by mm_lo, then quadrant signs
            v.tensor_mul(acc[:], w[:], p[:])        # mm_lo * cos
            v.tensor_add(acc[:], acc[:], sm[:])     # sin'
            v.tensor_mul(sm[:], w[:], sm[:])        # mm_lo * sin
            v.tensor_sub(p[:], p[:], sm[:])         # cos'
            nc.scalar.sign(s_out[:], nz[:])
            v.tensor_mul(s_out[:], s_out[:], acc[:])
            v.tensor_scalar(out=acc[:], in0=a[:], scalar1=-HALF_PI, scalar2=None,
                            op0=ALU.add)
            nc.scalar.sign(c_out[:], acc[:])
            v.tensor_mul(c_out[:], c_out[:], p[:])

        # ---- per-layer coefficient tiles, one partition per pair ----
        # groups: 0 = even pairs 0..127, 1 = even pairs 128..255,
        #         2 = odd pairs 0..127,  3 = odd pairs 128..255(pad)
        CO = {}
        for g, (thn, lon, ion) in enumerate(
            (("the0", "le0", "ie0"), ("the1", "le1", "ie1"),
             ("tho0", "lo0", "io0"), ("tho1", "lo1", "io1"))
        ):
            th = srcp.tile([128, L], F32, tag=f"th{g}")
            lo = srcp.tile([128, L], F32, tag=f"lo{g}")
            io = srcp.tile([128, L], F32, tag=f"io{g}")
            nc.sync.dma_start(out=th[:], in_=par[thn][:])
            nc.sync.dma_start(out=lo[:], in_=par[lon][:])
            nc.sync.dma_start(out=io[:], in_=par[ion][:])

            c_ = srcp.tile([128, L], F32, tag="c_")
            s_ = srcp.tile([128, L], F32, tag="s_")
            u_ = srcp.tile([128, L], F32, tag="u_")
            vp = srcp.tile([128, L], F32, tag="vp")
            vm = srcp.tile([128, L], F32, tag="vm")
            pp = srcp.tile([128, L], F32, tag="pp")
            pm = srcp.tile([128, L], F32, tag="pm")

            # clamp theta to [0, 2pi] (STE clamp forward value)
            nc.vector.tensor_scalar(out=th[:], in0=th[:], scalar1=0.0,
                                    scalar2=TWO_PI, op0=ALU.max, op1=ALU.min)
            cos_sin(c_, s_, th, "cs")
            # u = 1 - loss ; vp = 0.5 + imb ; vm = 0.5 - imb
            nc.vector.tensor_scalar(out=u_[:], in0=lo[:], scalar1=-1.0,
                                    scalar2=1.0, op0=ALU.mult, op1=ALU.add)
            nc.vector.tensor_scalar(out=vp[:], in0=io[:], scalar1=0.5,
                                    scalar2=None, op0=ALU.add)
            nc.vector.tensor_scalar(out=vm[:], in0=io[:], scalar1=-1.0,
                                    scalar2=0.5, op0=ALU.mult, op1=ALU.add)
            nc.vector.tensor_mul(pp[:], u_[:], vp[:])
            nc.vector.tensor_mul(pm[:], u_[:], vm[:])

            # one packed tile per group: coeff index  0:t 1:r 2:A 3:B 4:Bn 5:rn 6:E 7:En 8:Dn
            cg = coefp.tile([128, 9, L], F32, tag=f"cg{g}")
            t_ = cg[:, 0, :]; r_ = cg[:, 1, :]; A_ = cg[:, 2, :]
            B_ = cg[:, 3, :]; Bn = cg[:, 4, :]; rn = cg[:, 5, :]
            E_ = cg[:, 6, :]; En = cg[:, 7, :]; Dn = cg[:, 8, :]
            # ACT Sqrt is low precision (~1e-4); one Newton step fixes it:
            # y = 0.5*(y0 + x/y0), guarded against x=0 (odd pad pair has r=0).
            def sqrt_ref(dst, x, y0t, rec):
                nc.scalar.activation(y0t[:], x[:], ACTF.Sqrt)
                nc.vector.tensor_scalar(out=rec[:], in0=y0t[:], scalar1=1e-30,
                                        scalar2=None, op0=ALU.max)
                nc.vector.reciprocal(rec[:], rec[:])
                nc.vector.tensor_mul(rec[:], x[:], rec[:])
                nc.vector.tensor_add(rec[:], rec[:], y0t[:])
                nc.vector.tensor_scalar(out=dst, in0=rec[:], scalar1=0.5,
                                        scalar2=None, op0=ALU.mult)

            sq_y0 = srcp.tile([128, L], F32, tag="sqy")
            sq_rc = srcp.tile([128, L], F32, tag="sqr")
            sqrt_ref(t_, pp, sq_y0, sq_rc)
            sqrt_ref(r_, pm, sq_y0, sq_rc)
            nc.vector.tensor_mul(A_, t_, c_[:])
            nc.vector.tensor_mul(B_, t_, s_[:])
            nc.vector.tensor_mul(E_, r_, c_[:])
            # reuse pp as D = r*s scratch
            nc.vector.tensor_mul(pp[:], r_, s_[:])
            nc.vector.tensor_scalar(out=Dn, in0=pp[:], scalar1=-1.0,
                                    scalar2=None, op0=ALU.mult)
            nc.vector.tensor_scalar(out=Bn, in0=B_, scalar1=-1.0,
                                    scalar2=None, op0=ALU.mult)
            nc.vector.tensor_scalar(out=rn, in0=r_, scalar1=-1.0,
                                    scalar2=None, op0=ALU.mult)
            nc.vector.tensor_scalar(out=En, in0=E_, scalar1=-1.0,
                                    scalar2=None, op0=ALU.mult)
            CO[g] = cg

        # ---- state ----
        Ea = stp.tile([128, 2, 2, CPD], F32, tag="Ea")
        Oa = stp.tile([128, 2, 2, CPD], F32, tag="Oa")
        Eb = stp.tile([128, 2, 2, CPD], F32, tag="Eb")
        Ob = stp.tile([128, 2, 2, CPD], F32, tag="Ob")
        EsA = stp.tile([128, 2, 2, CPD], F32, tag="EsA")
        EsB = stp.tile([128, 2, 2, CPD], F32, tag="EsB")

        nc.vector.memset(Ea[:], 0.0)
        nc.vector.memset(Oa[:], 0.0)
        nc.sync.dma_start(out=Ea[:, :, 0, :], in_=par["init_e"][:])
        nc.sync.dma_start(out=Oa[:, :, 0, :], in_=par["init_o"][:])

        SU = shifts[:, 0, :]
        SB = shifts[:, 1, :]
        SD = shifts[:, 2, :]
        S00 = shifts[:, 3, :]
        SB127 = shifts[:, 4, :]

        CIDX = dict(t=0, r=1, A=2, B=3, Bn=4, rn=5, E=6, En=7, Dn=8)

        def mix(dst, srcT, srcB, stage, li):
            """One PC+MMI column: top rows srcT, bottom rows srcB -> dst.

            dst/srcT/srcB: dicts with APs xt,yt,xb,yb (each [128, CPD]).
            stage: [128, 9, 2U] staged coeffs; li: static in-stage index.
            """
            C = {k: stage[:, v, :] for k, v in CIDX.items()}

            def sl(T):
                return T[:, li : li + 1]

            v = nc.vector
            g_ = nc.gpsimd

            def lead_act(out, in_, coef):
                nc.scalar.activation(out, in_, ACTF.Copy, bias=0.0, scale=sl(coef))

            def stt(eng, out, in0, coef, in1):
                eng.scalar_tensor_tensor(out=out, in0=in0, scalar=sl(coef),
                                         in1=in1, op0=ALU.mult, op1=ALU.add)

            # Engine split: ACT takes the xt/yt leading multiplies, GpSimd the
            # merged bottom lead t*[xb|yb] (one [128,128] tensor_scalar; GpSimd
            # has no scalar_tensor_tensor opcode), DVE the 8 fused mul-adds.
            lead_act(dst["xt"], srcT["xt"], C["A"])
            lead_act(dst["yt"], srcT["xt"], C["B"])
            if "xyb" in srcB:
                lead_act(dst["xyb"], srcB["xyb"], C["t"])
            else:
                lead_act(dst["xb"], srcB["xb"], C["t"])
                lead_act(dst["yb"], srcB["yb"], C["t"])
            # X_top' = A xt + Bn yt + rn yb
            stt(v, dst["xt"], srcT["yt"], C["Bn"], dst["xt"])
            stt(v, dst["xt"], srcB["yb"], C["rn"], dst["xt"])
            # Y_top' = B xt + A yt + r xb
            stt(v, dst["yt"], srcT["yt"], C["A"], dst["yt"])
            stt(v, dst["yt"], srcB["xb"], C["r"], dst["yt"])
            # X_bot' = t xb + Dn xt + En yt
            stt(v, dst["xb"], srcT["xt"], C["Dn"], dst["xb"])
            stt(v, dst["xb"], srcT["yt"], C["En"], dst["xb"])
            # Y_bot' = t yb + E xt + Dn yt
            stt(v, dst["yb"], srcT["xt"], C["E"], dst["yb"])
            stt(v, dst["yb"], srcT["yt"], C["Dn"], dst["yb"])

        def even_layer(srcE, srcO, dstE, dstO, stages, li):
            for t in (0, 1):
                mix(
                    dict(xt=dstE[:, t, 0, :], yt=dstE[:, t, 1, :],
                         xb=dstO[:, t, 0, :], yb=dstO[:, t, 1, :],
                         xyb=dstO[:, t, :, :]),
                    dict(xt=srcE[:, t, 0, :], yt=srcE[:, t, 1, :]),
                    dict(xb=srcO[:, t, 0, :], yb=srcO[:, t, 1, :],
                         xyb=srcO[:, t, :, :]),
                    stages[t], li,
                )

        def odd_layer(srcO, botX, botY, dstO, dstEs, stages, li, botXY=None):
            # top = O[k], bottom = E[k+1] (pre-shifted into botX/botY APs)
            for t in (0, 1):
                mix(
                    dict(xt=dstO[:, t, 0, :], yt=dstO[:, t, 1, :],
                         xb=dstEs[:, t, 0, :], yb=dstEs[:, t, 1, :],
                         xyb=dstEs[:, t, :, :]),
                    dict(xt=srcO[:, t, 0, :], yt=srcO[:, t, 1, :]),
                    dict(xb=botX(t), yb=botY(t), xyb=botXY(t)),
                    stages[2 + t], li,
                )

        U = 4  # scan steps per loop iteration

        def body(j, u_steps=None):
            # j = base even/odd-layer index for this iteration (advances by 2U).
            # One dynamic-sliced copy per coeff group, then all static slices
            # (dynamic APs burn engine address registers: ~24/body max).
            if u_steps is None:
                u_steps = U
            stages = []
            for g in range(4):
                sg = stgp.tile([128, 9, 2 * U], F32, tag=f"stage{g}")
                nc.vector.tensor_copy(out=sg[:], in_=CO[g][:, :, bass.ds(j, 2 * U)])
                stages.append(sg)
            for u in range(u_steps):
                li0, li1 = 2 * u, 2 * u + 1
                even_layer(Ea, Oa, Eb, Ob, stages, li0)
                even_layer(Eb, Ob, Ea, Oa, stages, li1)

                # Esh[k] = E[k+1]  (linear over the two E tiles), built on PE
                psh = psp.tile([128, 2, 2, CPD], F32, tag="psh")
                nc.tensor.matmul(out=psh[:, 1, :, :], lhsT=SU, rhs=Ea[:, 1, :, :],
                                 start=True, stop=True)
                nc.tensor.matmul(out=psh[:, 0, :, :], lhsT=SU, rhs=Ea[:, 0, :, :],
                                 start=True, stop=False)
                nc.tensor.matmul(out=psh[:, 0, :, :], lhsT=SB, rhs=Ea[:, 1, :, :],
                                 start=False, stop=True)
                # PSUM -> SBUF so GpSimd chains can read it (and DVE avoids
                # the PSUM-source penalty)
                esh = stgp.tile([128, 2, 2, CPD], F32, tag="esh")
                nc.scalar.copy(out=esh[:], in_=psh[:])

                odd_layer(Oa, lambda t: esh[:, t, 0, :], lambda t: esh[:, t, 1, :],
                          Ob, EsB, stages, li0, botXY=lambda t: esh[:, t, :, :])
                odd_layer(Ob, lambda t: EsB[:, t, 0, :], lambda t: EsB[:, t, 1, :],
                          Oa, EsA, stages, li1, botXY=lambda t: EsB[:, t, :, :])

                # shift Es back: E'[k+1] = Es[k]; E'[0] = old E[0] (row 0 fixed)
                peb = psp.tile([128, 2, 2, CPD], F32, tag="peb")
                nc.tensor.matmul(out=peb[:, 0, :, :], lhsT=SD, rhs=EsA[:, 0, :, :],
                                 start=True, stop=False)
                nc.tensor.matmul(out=peb[:, 0, :, :], lhsT=S00, rhs=Ea[:, 0, :, :],
                                 start=False, stop=True)
                nc.tensor.matmul(out=peb[:, 1, :, :], lhsT=SD, rhs=EsA[:, 1, :, :],
                                 start=True, stop=False)
                nc.tensor.matmul(out=peb[:, 1, :, :], lhsT=SB127, rhs=EsA[:, 0, :, :],
                                 start=False, stop=True)
                nc.scalar.copy(out=Ea[:], in_=peb[:])

        if n_steps > 2:
            assert (2 * n_steps) % (2 * U) == 0
            with tc.For_i(0, n_steps * 2, 2 * U) as j:
                body(j)
        else:
            for k in range(n_steps):
                body(2 * k, u_steps=1)

        # ---- output phases and store ----
        po = consts.tile([128, 4], F32, tag="po")
        co = consts.tile([128, 4], F32, tag="co")
        so = consts.tile([128, 4], F32, tag="so")
        son = consts.tile([128, 4], F32, tag="son")
        nc.sync.dma_start(out=po[:], in_=par["pout"][:])
        nc.vector.tensor_scalar(out=po[:], in0=po[:], scalar1=0.0,
                                scalar2=TWO_PI, op0=ALU.max, op1=ALU.min)
        cos_sin(co, so, po, "csout")
        nc.vector.tensor_scalar(out=son[:], in0=so[:], scalar1=-1.0,
                                scalar2=None, op0=ALU.mult)

        fE = stp.tile([128, 2, 2, CPD], F32, tag="fE")
        fO = stp.tile([128, 2, 2, CPD], F32, tag="fO")
        for (S, D, c0) in ((Ea, fE, 0), (Oa, fO, 2)):
            for t in (0, 1):
                cs = co[:, c0 + t : c0 + t + 1]
                ss = so[:, c0 + t : c0 + t + 1]
                sn = son[:, c0 + t : c0 + t + 1]
                v = nc.vector
                v.tensor_scalar(out=D[:, t, 0, :], in0=S[:, t, 0, :],
                                scalar1=cs, scalar2=None, op0=ALU.mult)
                v.scalar_tensor_tensor(out=D[:, t, 0, :], in0=S[:, t, 1, :],
                                       scalar=sn, in1=D[:, t, 0, :],
                                       op0=ALU.mult, op1=ALU.add)
                v.tensor_scalar(out=D[:, t, 1, :], in0=S[:, t, 1, :],
                                scalar1=cs, scalar2=None, op0=ALU.mult)
                v.scalar_tensor_tensor(out=D[:, t, 1, :], in0=S[:, t, 0, :],
                                       scalar=ss, in1=D[:, t, 1, :],
                                       op0=ALU.mult, op1=ALU.add)
        nc.sync.dma_start(out=outv[:, 0, :, :, :], in_=fE[:])
        nc.sync.dma_start(out=outv[:, 1, :, :, :], in_=fO[:])

    nc.finalize()
    return nc


def _host_inputs(pc_even_phases, pc_odd_phases, pc_out_phases,
                 mmi_loss_even, mmi_imb_even, mmi_loss_odd, mmi_imb_odd,
                 n_steps=L // 2):
    f = np.float32
    thT = np.ascontiguousarray(pc_even_phases.T.astype(f))      # [256, 512]
    leT = np.ascontiguousarray(mmi_loss_even.T.astype(f))
    ieT = np.ascontiguousarray(mmi_imb_even.T.astype(f))

    tho = np.zeros((256, L), f)
    loo = np.zeros((256, L), f)
    ioo = np.zeros((256, L), f)
    tho[:255] = pc_odd_phases.T
    loo[:255] = mmi_loss_odd.T
    ioo[:255] = mmi_imb_odd.T
    ioo[255] = 0.5  # pad pair -> identity (t=1, r=0)

    shifts = np.zeros((128, 5, 128), f)
    for p in range(127):
        shifts[p + 1, 0, p] = 1.0     # SU: out[p] = in[p+1]
        shifts[p, 2, p + 1] = 1.0     # SD: out[p+1] = in[p]
    shifts[0, 1, 127] = 1.0           # SB: out[127] = in[0]
    shifts[0, 3, 0] = 1.0             # S00: out[0] = in[0]
    shifts[127, 4, 0] = 1.0           # SB127: out[0] = in[127]

    pout = np.zeros((128, 4), f)
    p = np.arange(128)
    pc = pc_out_phases.astype(f)
    pout[:, 0] = pc[2 * p]
    pout[:, 1] = pc[256 + 2 * p]
    pout[:, 2] = pc[2 * p + 1]
    pout[:, 3] = pc[257 + 2 * p]

    base = {
        "the0": thT[:128], "the1": thT[128:],
        "le0": leT[:128], "le1": leT[128:],
        "ie0": ieT[:128], "ie1": ieT[128:],
        "tho0": tho[:128], "tho1": tho[128:],
        "lo0": loo[:128], "lo1": loo[128:],
        "io0": ioo[:128], "io1": ioo[128:],
        "pout": pout, "shifts": shifts,
    }

    in_maps = []
    for d in range(NCORES):
        init_e = np.zeros((128, 2, CPD), f)
        init_o = np.zeros((128, 2, CPD), f)
        for j in range(CPD):
            row = CPD * d + j
            t, rr = divmod(row, 256)
            if rr % 2 == 0:
                init_e[rr // 2, t, j] = 1.0
            else:
                init_o[(rr - 1) // 2, t, j] = 1.0
        m = dict(base)
        m["init_e"] = init_e
        m["init_o"] = init_o
        in_maps.append(m)
    return in_maps


def _assemble(results):
    M = np.zeros((N, N), np.complex64)
    p = np.arange(128)
    for d in range(NCORES):
        o = results[d]["outv"]  # [128, 2(E/O), 2(X/Y), 2(t), CPD]
        cols = slice(CPD * d, CPD * (d + 1))
        for t in (0, 1):
            rE = 256 * t + 2 * p
            rO = 256 * t + 2 * p + 1
            M[rE, cols] = o[:, 0, t, 0, :] + 1j * o[:, 0, t, 1, :]
            M[rO, cols] = o[:, 1, t, 0, :] + 1j * o[:, 1, t, 1, :]
    return M


def _run(inputs, trace=False):
    if "nc" not in _CACHE:
        _CACHE["nc"] = _build_program()
    nc = _CACHE["nc"]
    inputs = {k: np.asarray(v) for k, v in inputs.items()}
    in_maps = _host_inputs(**inputs)
    try:
        res = run_bass_kernel_spmd(nc, in_maps, list(range(NCORES)), trace=trace)
    except Exception:
        # transient NRT_EXEC_UNIT_UNRECOVERABLE hiccups resolve on retry
        import time
        time.sleep(20)
        res = run_bass_kernel_spmd(nc, in_maps, list(range(NCORES)), trace=trace)
    return _assemble(res.results), res


def kernel(**inputs):
    return _run(inputs)[0]



# revision 2
# speedup vs baseline: 1.0716x; 1.0716x over previous
"""Two-phase Trainium2 kernel for the Clements mesh (N=512, 1024 layer apps).

Phase A (launch LA): each of the 8 cores builds 4 "leaf" products of 32
consecutive layer applications as banded 512x512 complex matrices (band
+-16) in E/O-parity band layout, using per-partition scalar mixes.
Phase B (launches LB1..LE): a binary matmul tree combines the 32 leaves
into the full transfer matrix on the TensorEngine; the host only
re-slices/transposes/zero-pads blocks between launches.
"""

import sys

sys.path.insert(0, "/opt/trn_rl_repo")

from contextlib import ExitStack

import numpy as np

import concourse.bass as bass
import concourse.tile as tile
from concourse import bacc, mybir
from concourse.bass_utils import run_bass_kernel_spmd

F32 = mybir.dt.float32
ALU = mybir.AluOpType
ACTF = mybir.ActivationFunctionType

N = 512
L = 512
TWO_PI = 6.283185307179586
HALF_PI = 1.5707963267948966
NCORES = 8

W = 40        # band tile width
A0 = 19       # diagonal position inside the band tile
NLEAF_C = 4   # leaves per core
STEPS = 8     # scan steps per leaf (32 layer applications)
WLO, WHI = 3, 36   # active band window (max half-width 16)

_CACHE = {}


# --------------------------------------------------------------------------
# Launch A: band build
# --------------------------------------------------------------------------

def _build_banded():
    nc = bacc.Bacc("TRN2", target_bir_lowering=False)

    par = {}
    # 64 even + 64 odd layer apps per core; columns ordered [step s: 16][leaf
    # q: 4][j: 2] so one dynamic slice per step grabs all 4 leaves' coeffs.
    for nm in ("the0", "the1", "le0", "le1", "ie0", "ie1",
               "tho0", "tho1", "lo0", "lo1", "io0", "io1"):
        par[nm] = nc.declare_dram_parameter(nm, [128, 64], F32, isOutput=False)
    par["pout"] = nc.declare_dram_parameter("pout", [128, 4], F32, isOutput=False)
    par["shifts"] = nc.declare_dram_parameter("shifts", [128, 5, 128], F32, isOutput=False)
    outE = nc.declare_dram_parameter("outE", [128, NLEAF_C, 2, 2, W], F32, isOutput=True)
    outO = nc.declare_dram_parameter("outO", [128, NLEAF_C, 2, 2, W], F32, isOutput=True)

    with tile.TileContext(nc) as tc, ExitStack() as ctx:
        consts = ctx.enter_context(tc.tile_pool(name="consts", bufs=1))
        coefp = ctx.enter_context(tc.tile_pool(name="coefs", bufs=1))
        srcp = ctx.enter_context(tc.tile_pool(name="srcs", bufs=1))
        stp = ctx.enter_context(tc.tile_pool(name="state", bufs=1))
        stgp = ctx.enter_context(tc.tile_pool(name="stage", bufs=2))
        psp = ctx.enter_context(tc.tile_pool(name="psum", bufs=1, space="PSUM"))

        shifts = consts.tile([128, 5, 128], F32, tag="shifts")
        nc.sync.dma_start(out=shifts[:], in_=par["shifts"][:])

        SINP = (-0.16666666639369604, 0.0083333316715976, -0.00019840942043806986,
                2.752917460996653e-06, -2.3955613511594512e-08)
        COSP = (-0.49999999647064386, 0.041666645176626854, -0.0013888464831511677,
                2.4765157753536994e-05, -2.6136488530828197e-07)
        PI_HI = 3.1415927410125732
        PI_LO = -8.742278012618954e-08

        def cos_sin(c_out, s_out, th, tagp, v=None):
            shape = list(th.shape)
            t_ = lambda nm: srcp.tile(shape, F32, tag=f"{tagp}{nm}", name=f"{tagp}{nm}")
            z, nz, a, mm, m2, p, acc, msk, w_, sm = (
                t_(n) for n in ("z", "nz", "a", "mm", "m2", "p", "acc", "msk", "w", "sm"))
            v = v or nc.vector
            v.tensor_scalar(out=z[:], in0=th[:], scalar1=-PI_HI, scalar2=None, op0=ALU.add)
            v.tensor_scalar(out=nz[:], in0=z[:], scalar1=-1.0, scalar2=None, op0=ALU.mult)
            v.tensor_tensor(out=a[:], in0=nz[:], in1=z[:], op=ALU.max)
            v.tensor_scalar(out=mm[:], in0=a[:], scalar1=-1.0, scalar2=PI_HI,
                            op0=ALU.mult, op1=ALU.add)
            v.tensor_scalar(out=msk[:], in0=mm[:], scalar1=1.0, scalar2=None,
                            op0=ALU.bypass)
            v.tensor_tensor(out=mm[:], in0=mm[:], in1=a[:], op=ALU.min)
            v.tensor_tensor(out=msk[:], in0=a[:], in1=msk[:], op=ALU.is_gt)
            nc.scalar.sign(w_[:], z[:])
            v.tensor_scalar(out=acc[:], in0=msk[:], scalar1=2.0, scalar2=-1.0,
                            op0=ALU.mult, op1=ALU.add)
            v.tensor_mul(w_[:], w_[:], acc[:])
            v.tensor_add(w_[:], w_[:], msk[:])
            v.tensor_scalar(out=w_[:], in0=w_[:], scalar1=PI_LO, scalar2=None, op0=ALU.mult)
            v.tensor_mul(m2[:], mm[:], mm[:])
            v.tensor_scalar(out=p[:], in0=m2[:], scalar1=SINP[4], scalar2=SINP[3],
                            op0=ALU.mult, op1=ALU.add)
            for cf in (SINP[2], SINP[1], SINP[0]):
                v.tensor_mul(p[:], p[:], m2[:])
                v.tensor_scalar(out=p[:], in0=p[:], scalar1=cf, scalar2=None, op0=ALU.add)
            v.tensor_mul(acc[:], m2[:], mm[:])
            v.tensor_mul(p[:], p[:], acc[:])
            v.tensor_add(sm[:], p[:], mm[:])
            v.tensor_scalar(out=p[:], in0=m2[:], scalar1=COSP[4], scalar2=COSP[3],
                            op0=ALU.mult, op1=ALU.add)
            for cf in (COSP[2], COSP[1], COSP[0]):
                v.tensor_mul(p[:], p[:], m2[:])
                v.tensor_scalar(out=p[:], in0=p[:], scalar1=cf, scalar2=None, op0=ALU.add)
            v.tensor_mul(p[:], p[:], m2[:])
            v.tensor_scalar(out=p[:], in0=p[:], scalar1=1.0, scalar2=None, op0=ALU.add)
            v.tensor_mul(acc[:], w_[:], p[:])
            v.tensor_add(acc[:], acc[:], sm[:])
            v.tensor_mul(sm[:], w_[:], sm[:])
            v.tensor_sub(p[:], p[:], sm[:])
            nc.scalar.sign(s_out[:], nz[:])
            v.tensor_mul(s_out[:], s_out[:], acc[:])
            v.tensor_scalar(out=acc[:], in0=a[:], scalar1=-HALF_PI, scalar2=None,
                            op0=ALU.add)
            nc.scalar.sign(c_out[:], acc[:])
            v.tensor_mul(c_out[:], c_out[:], p[:])

        # ---- per-layer coefficients: groups 0/1 = even pairs lo/hi,
        #      2/3 = odd pairs lo/hi; packed idx 0:t 1:r 2:A 3:B 4:Bn 5:rn
        #      6:E 7:En 8:Dn
        CO = {}
        for g, (thn, lon, ion) in enumerate(
            (("the0", "le0", "ie0"), ("the1", "le1", "ie1"),
             ("tho0", "lo0", "io0"), ("tho1", "lo1", "io1"))
        ):
            th = srcp.tile([128, 64], F32, tag=f"th{g}")
            lo = srcp.tile([128, 64], F32, tag=f"lo{g}")
            io = srcp.tile([128, 64], F32, tag=f"io{g}")
            nc.sync.dma_start(out=th[:], in_=par[thn][:])
            nc.sync.dma_start(out=lo[:], in_=par[lon][:])
            nc.sync.dma_start(out=io[:], in_=par[ion][:])

            ve = nc.vector
            u_ = srcp.tile([128, 64], F32, tag=f"u{g % 2}_")
            vp = srcp.tile([128, 64], F32, tag=f"vp{g % 2}")
            vm = srcp.tile([128, 64], F32, tag=f"vm{g % 2}")
            pp = srcp.tile([128, 64], F32, tag=f"pp{g % 2}")
            pm = srcp.tile([128, 64], F32, tag=f"pm{g % 2}")

            # packed coefficients: 0:t 1:r 2:rn 3:tc 4:ts 5:tsn 6:rt 7:rtn
            cg = coefp.tile([128, 8, 64], F32, tag=f"cg{g}")
            t_ = cg[:, 0, :]; r_ = cg[:, 1, :]; rn = cg[:, 2, :]
            tc = cg[:, 3, :]; ts_ = cg[:, 4, :]; tsn = cg[:, 5, :]
            rt = cg[:, 6, :]; rtn = cg[:, 7, :]
            cc = srcp.tile([128, 64], F32, tag=f"cc{g % 2}")
            ss = srcp.tile([128, 64], F32, tag=f"ss{g % 2}")

            ve.tensor_scalar(out=th[:], in0=th[:], scalar1=0.0,
                             scalar2=TWO_PI, op0=ALU.max, op1=ALU.min)
            cos_sin(cc, ss, th, f"cs{g % 2}", v=ve)
            ve.tensor_scalar(out=u_[:], in0=lo[:], scalar1=-1.0,
                             scalar2=1.0, op0=ALU.mult, op1=ALU.add)
            ve.tensor_scalar(out=vp[:], in0=io[:], scalar1=0.5,
                             scalar2=None, op0=ALU.add)
            ve.tensor_scalar(out=vm[:], in0=io[:], scalar1=-1.0,
                             scalar2=0.5, op0=ALU.mult, op1=ALU.add)
            ve.tensor_mul(pp[:], u_[:], vp[:])
            ve.tensor_mul(pm[:], u_[:], vm[:])

            def sqrt_ref(dst, x, y0t, rec):
                nc.scalar.activation(y0t[:], x[:], ACTF.Sqrt)
                ve.tensor_scalar(out=rec[:], in0=y0t[:], scalar1=1e-30,
                                 scalar2=None, op0=ALU.max)
                nc.vector.reciprocal(rec[:], rec[:])
                ve.tensor_mul(rec[:], x[:], rec[:])
                ve.tensor_add(rec[:], rec[:], y0t[:])
                ve.tensor_scalar(out=dst, in0=rec[:], scalar1=0.5,
                                 scalar2=None, op0=ALU.mult)

            sq_y0 = srcp.tile([128, 64], F32, tag=f"sqy{g % 2}")
            sq_rc = srcp.tile([128, 64], F32, tag=f"sqr{g % 2}")
            sqrt_ref(t_, pp, sq_y0, sq_rc)
            sqrt_ref(r_, pm, sq_y0, sq_rc)
            ve.tensor_scalar(out=rn, in0=r_, scalar1=-1.0,
                             scalar2=None, op0=ALU.mult)
            ve.tensor_mul(tc, t_, cc[:])
            ve.tensor_mul(ts_, t_, ss[:])
            ve.tensor_scalar(out=tsn, in0=ts_, scalar1=-1.0,
                             scalar2=None, op0=ALU.mult)
            nc.vector.reciprocal(u_[:], t_)       # 1/t  (t >= 0.65 by input range)
            ve.tensor_mul(rt, r_, u_[:])
            ve.tensor_scalar(out=rtn, in0=rt, scalar1=-1.0,
                             scalar2=None, op0=ALU.mult)
            CO[g] = cg

        # ---- per-leaf state ----
        ST = []
        for q in range(NLEAF_C):
            tiles = {}
            for nm in ("Ea", "Oa", "Eb", "Ob", "EsA", "EsB", "tmp"):
                t = stp.tile([128, 2, 2, W], F32, tag=f"{nm}{q}")
                nc.vector.memset(t[:], 0.0)
                tiles[nm] = t
            nc.vector.memset(tiles["Ea"][:, :, 0, A0:A0 + 1], 1.0)
            nc.vector.memset(tiles["Oa"][:, :, 0, A0:A0 + 1], 1.0)
            ST.append(tiles)

        SU = shifts[:, 0, :]
        SB = shifts[:, 1, :]
        SD = shifts[:, 2, :]
        S00 = shifts[:, 3, :]
        SB127 = shifts[:, 4, :]

        CIDX = dict(t=0, r=1, rn=2, tc=3, ts=4, tsn=5, rt=6, rtn=7)

        def mix(dst, srcT, srcB, tmp, grp, ci):
            """One PC+MMI band layer.  tmp = t*phase*top (2 ACT + 2 DVE);
            top outs = tmp + i r*bot[d-1] (2 DVE, in1=tmp); bottom =
            t*bot (1 GpSimd) + i (r/t)*tmp[d+1] (2 DVE)."""
            C = {k: CO[grp][:, v, ci:ci + 1] for k, v in CIDX.items()}
            v, g_, a_ = nc.vector, nc.gpsimd, nc.scalar

            def stt(out, in0, coef, in1):
                v.scalar_tensor_tensor(out=out, in0=in0, scalar=coef,
                                       in1=in1, op0=ALU.mult, op1=ALU.add)

            a_.activation(tmp["xt"], srcT["xt"], ACTF.Copy, bias=0.0, scale=C["tc"])
            a_.activation(tmp["yt"], srcT["xt"], ACTF.Copy, bias=0.0, scale=C["ts"])
            stt(tmp["xt"], srcT["yt"], C["tsn"], tmp["xt"])
            stt(tmp["yt"], srcT["yt"], C["tc"], tmp["yt"])
            g_.tensor_scalar(out=dst["xyb"], in0=srcB["xyb"], scalar1=C["t"],
                             scalar2=None, op0=ALU.mult)
            stt(dst["xt"], srcB["yb_m"], C["rn"], tmp["xt"])
            stt(dst["yt"], srcB["xb_m"], C["r"], tmp["yt"])
            stt(dst["xb"], tmp["yt_p"], C["rtn"], dst["xb"])
            stt(dst["yb"], tmp["xt_p"], C["rt"], dst["yb"])

        def win(s):
            h = 2 * s + 2
            return slice(A0 - h, A0 + h + 1)

        def aps_top(T, t, w):
            return dict(xt=T[:, t, 0, w], yt=T[:, t, 1, w])

        def aps_tmp(T, t, w, wp):
            return dict(xt=T[:, t, 0, w], yt=T[:, t, 1, w], xyt=T[:, t, :, w],
                        xt_p=T[:, t, 0, wp], yt_p=T[:, t, 1, wp])

        def aps_bot(T, t, w, wm):
            return dict(xb=T[:, t, 0, w], yb=T[:, t, 1, w],
                        xb_m=T[:, t, 0, wm], yb_m=T[:, t, 1, wm],
                        xyb=T[:, t, :, w])

        def aps_dst(TT, TB, t, w):
            return dict(xt=TT[:, t, 0, w], yt=TT[:, t, 1, w], xyt=TT[:, t, :, w],
                        xb=TB[:, t, 0, w], yb=TB[:, t, 1, w], xyb=TB[:, t, :, w])

        EV_SRC = ((("Ea", "Oa"), ("Eb", "Ob")), (("Eb", "Ob"), ("Ea", "Oa")))
        OD_SRC = ((("Oa", "esh"), ("Ob", "EsB")), (("Ob", "EsB"), ("Oa", "EsA")))

        for s in range(STEPS):
            w = win(s)
            wm = slice(w.start - 1, w.stop - 1)
            wp = slice(w.start + 1, w.stop + 1)
            wx = slice(w.start - 1, w.stop + 1)
            ci0 = 8 * s

            # two even layers, interleaved across leaves
            for jj in (0, 1):
                for q in range(NLEAF_C):
                    (sE, sO), (dE, dO) = EV_SRC[jj]
                    S = ST[q]
                    for t in (0, 1):
                        mix(aps_dst(S[dE], S[dO], t, w),
                            aps_top(S[sE], t, w),
                            aps_bot(S[sO], t, w, wm),
                            aps_tmp(S["tmp"], t, w, wp), t, ci0 + 2 * q + jj)

            # Esh[k] = E[k+1] for all leaves
            for q in range(NLEAF_C):
                Ea = ST[q]["Ea"]
                psh = psp.tile([128, 2, 2, W], F32, tag=f"psh{q}")
                nc.tensor.matmul(out=psh[:, 1, :, :], lhsT=SU, rhs=Ea[:, 1, :, :],
                                 start=True, stop=True)
                nc.tensor.matmul(out=psh[:, 0, :, :], lhsT=SU, rhs=Ea[:, 0, :, :],
                                 start=True, stop=False)
                nc.tensor.matmul(out=psh[:, 0, :, :], lhsT=SB, rhs=Ea[:, 1, :, :],
                                 start=False, stop=True)
                esh = stgp.tile([128, 2, 2, W], F32, tag=f"esh{q}")
                nc.scalar.copy(out=esh[:, :, :, wx], in_=psh[:, :, :, wx])
                ST[q]["esh"] = esh

            # two odd layers
            for jj in (0, 1):
                for q in range(NLEAF_C):
                    (sO, sB), (dO, dEs) = OD_SRC[jj]
                    S = ST[q]
                    for t in (0, 1):
                        mix(aps_dst(S[dO], S[dEs], t, w),
                            aps_top(S[sO], t, w),
                            aps_bot(S[sB], t, w, wm),
                            aps_tmp(S["tmp"], t, w, wp), 2 + t, ci0 + 2 * q + jj)

            # unshift: E'[k+1] = Es[k]; E'[0] = old E[0]
            for q in range(NLEAF_C):
                Ea, EsA = ST[q]["Ea"], ST[q]["EsA"]
                peb = psp.tile([128, 2, 2, W], F32, tag=f"peb{q}")
                nc.tensor.matmul(out=peb[:, 0, :, :], lhsT=SD, rhs=EsA[:, 0, :, :],
                                 start=True, stop=False)
                nc.tensor.matmul(out=peb[:, 0, :, :], lhsT=S00, rhs=Ea[:, 0, :, :],
                                 start=False, stop=True)
                nc.tensor.matmul(out=peb[:, 1, :, :], lhsT=SD, rhs=EsA[:, 1, :, :],
                                 start=True, stop=False)
                nc.tensor.matmul(out=peb[:, 1, :, :], lhsT=SB127, rhs=EsA[:, 0, :, :],
                                 start=False, stop=True)
                nc.scalar.copy(out=Ea[:, :, :, wx], in_=peb[:, :, :, wx])

        # ---- output phase rotation on leaf 3 (identity unless core 7) ----
        po = consts.tile([128, 4], F32, tag="po")
        co = consts.tile([128, 4], F32, tag="co")
        so = consts.tile([128, 4], F32, tag="so")
        son = consts.tile([128, 4], F32, tag="son")
        nc.sync.dma_start(out=po[:], in_=par["pout"][:])
        nc.vector.tensor_scalar(out=po[:], in0=po[:], scalar1=0.0,
                                scalar2=TWO_PI, op0=ALU.max, op1=ALU.min)
        cos_sin(co, so, po, "csout")
        nc.vector.tensor_scalar(out=son[:], in0=so[:], scalar1=-1.0,
                                scalar2=None, op0=ALU.mult)

        lastE, lastO = ST[3]["Ea"], ST[3]["Oa"]
        fE = stp.tile([128, 2, 2, W], F32, tag="fE")
        fO = stp.tile([128, 2, 2, W], F32, tag="fO")
        nc.vector.memset(fE[:], 0.0)
        nc.vector.memset(fO[:], 0.0)
        for (Sx, D, c0) in ((lastE, fE, 0), (lastO, fO, 2)):
            for t in (0, 1):
                cs = co[:, c0 + t: c0 + t + 1]
                ss = so[:, c0 + t: c0 + t + 1]
                sn = son[:, c0 + t: c0 + t + 1]
                v = nc.vector
                v.tensor_scalar(out=D[:, t, 0, w], in0=Sx[:, t, 0, w],
                                scalar1=cs, scalar2=None, op0=ALU.mult)
                v.scalar_tensor_tensor(out=D[:, t, 0, w], in0=Sx[:, t, 1, w],
                                       scalar=sn, in1=D[:, t, 0, w],
                                       op0=ALU.mult, op1=ALU.add)
                v.tensor_scalar(out=D[:, t, 1, w], in0=Sx[:, t, 1, w],
                                scalar1=cs, scalar2=None, op0=ALU.mult)
                v.scalar_tensor_tensor(out=D[:, t, 1, w], in0=Sx[:, t, 0, w],
                                       scalar=ss, in1=D[:, t, 1, w],
                                       op0=ALU.mult, op1=ALU.add)

        for q in range(NLEAF_C):
            Esrc = fE if q == 3 else ST[q]["Ea"]
            Osrc = fO if q == 3 else ST[q]["Oa"]
            nc.sync.dma_start(out=outE[:, q, :, :, :], in_=Esrc[:])
            nc.sync.dma_start(out=outO[:, q, :, :, :], in_=Osrc[:])

    nc.finalize()
    return nc


def _host_inputs_banded(inp):
    """Per-core input maps for launch A."""
    f = np.float32

    def packed(arr, pad_val=None):
        # arr: [512, P] layer-major -> per-core [2, 128, 64] with columns
        # ordered [s:16][q:4][jj:2]; even/odd pair groups split by 128.
        a = arr.T.astype(f)                      # [P, 512]
        if a.shape[0] == 255:
            pad = np.zeros((1, 512), f)
            if pad_val is not None:
                pad[:] = pad_val
            a = np.concatenate([a, pad], axis=0)  # [256, 512]
        out = []
        for d in range(NCORES):
            cols = a[:, 64 * d:64 * (d + 1)]      # layer apps of core d
            # device column order [s:8][q:4][jj:2]; source col = 16q + 2s + jj
            idx = np.array([16 * q + 2 * s + jj
                            for s in range(8) for q in range(4)
                            for jj in range(2)], np.int64)
            out.append(cols[:, idx])
        return out  # list of [256, 64]

    the = packed(inp["pc_even_phases"])
    le = packed(inp["mmi_loss_even"])
    ie = packed(inp["mmi_imb_even"])
    tho = packed(inp["pc_odd_phases"])
    lo = packed(inp["mmi_loss_odd"])
    io = packed(inp["mmi_imb_odd"], pad_val=None)
    # odd pad pair 255 -> identity: t=1, r=0 needs imb=0.5, loss=0, th=0
    for d in range(NCORES):
        io[d][255, :] = 0.5

    shifts = np.zeros((128, 5, 128), f)
    for p in range(127):
        shifts[p + 1, 0, p] = 1.0     # SU: out[p] = in[p+1]
        shifts[p, 2, p + 1] = 1.0     # SD: out[p+1] = in[p]
    shifts[0, 1, 127] = 1.0           # SB: out[127](t0) = in[0](t1)
    shifts[0, 3, 0] = 1.0             # S00
    shifts[127, 4, 0] = 1.0           # SB127: out[0](t1) = in[127](t0)

    pout = np.zeros((128, 4), f)
    p = np.arange(128)
    pc = inp["pc_out_phases"].astype(f)
    pout[:, 0] = pc[2 * p]
    pout[:, 1] = pc[256 + 2 * p]
    pout[:, 2] = pc[2 * p + 1]
    pout[:, 3] = pc[257 + 2 * p]

    in_maps = []
    for d in range(NCORES):
        m = {
            "the0": the[d][:128], "the1": the[d][128:],
            "le0": le[d][:128], "le1": le[d][128:],
            "ie0": ie[d][:128], "ie1": ie[d][128:],
            "tho0": tho[d][:128], "tho1": tho[d][128:],
            "lo0": lo[d][:128], "lo1": lo[d][128:],
            "io0": io[d][:128], "io1": io[d][128:],
            "shifts": shifts,
            "pout": pout if d == 7 else np.zeros((128, 4), f),
        }
        in_maps.append(m)
    return in_maps


def _densify(E, O):
    """Band tiles [128,2,2,W] -> dense complex64 [512, 512]."""
    M = np.zeros((N, N), np.complex64)
    d = np.arange(W) - A0
    for t in (0, 1):
        p = np.arange(128)
        for (T, r) in ((E, 2 * (p + 128 * t)), (O, 2 * (p + 128 * t) + 1)):
            cols = r[:, None] + d[None, :]
            ok = (cols >= 0) & (cols < N)
            vals = T[:, t, 0, :] + 1j * T[:, t, 1, :]
            M[np.broadcast_to(r[:, None], cols.shape)[ok], cols[ok]] = vals[ok]
    return M


# --------------------------------------------------------------------------
# Generic block-matmul combine launch
# --------------------------------------------------------------------------

def _build_combine(G, T, F, name):
    """out[:, g, :] = sum_t lhsT[:, g, t, :].T @ rhs[:, g, t, :]."""
    nc = bacc.Bacc("TRN2", target_bir_lowering=False)
    lhsT = nc.declare_dram_parameter("lhsT", [128, G, T, 128], F32, isOutput=False)
    rhs = nc.declare_dram_parameter("rhs", [128, G, T, F], F32, isOutput=False)
    outv = nc.declare_dram_parameter("outv", [128, G, F], F32, isOutput=True)

    with tile.TileContext(nc) as tc, ExitStack() as ctx:
        lp = ctx.enter_context(tc.tile_pool(name="lhs", bufs=3))
        rp = ctx.enter_context(tc.tile_pool(name="rhs", bufs=3))
        op = ctx.enter_context(tc.tile_pool(name="out", bufs=3))
        pp = ctx.enter_context(tc.tile_pool(name="ps", bufs=4, space="PSUM"))

        for g in range(G):
            lt = lp.tile([128, T, 128], F32, tag="lt")
            rt = rp.tile([128, T, F], F32, tag="rt")
            nc.sync.dma_start(out=lt[:], in_=lhsT[:, g, :, :])
            nc.sync.dma_start(out=rt[:], in_=rhs[:, g, :, :])
            ps = pp.tile([128, F], F32, tag="ps")
            for t in range(T):
                nc.tensor.matmul(out=ps[:], lhsT=lt[:, t, :], rhs=rt[:, t, :],
                                 start=(t == 0), stop=(t == T - 1))
            ot = op.tile([128, F], F32, tag="ot")
            nc.scalar.copy(out=ot[:], in_=ps[:])
            nc.sync.dma_start(out=outv[:, g, :], in_=ot[:])

    nc.finalize()
    return nc


def _pad_block(Mre, Mim, rows, cols):
    """Zero-padded block [len(rows), len(cols)] from a dense matrix pair."""
    r0, r1 = rows
    c0, c1 = cols
    out_r = np.zeros((r1 - r0, c1 - c0), np.float32)
    out_i = np.zeros((r1 - r0, c1 - c0), np.float32)
    rr0, rr1 = max(r0, 0), min(r1, N)
    cc0, cc1 = max(c0, 0), min(c1, N)
    if rr1 > rr0 and cc1 > cc0:
        out_r[rr0 - r0:rr1 - r0, cc0 - c0:cc1 - c0] = Mre[rr0:rr1, cc0:cc1]
        out_i[rr0 - r0:rr1 - r0, cc0 - c0:cc1 - c0] = Mim[rr0:rr1, cc0:cc1]
    return out_r, out_i


def _pack_combine(jobs, G, T, F):
    """jobs: per core, list of G groups; each group = list of T (lhsT_blk,
    rhs_blk) fp32 pairs (128x128, 128xF).  Returns in_maps."""
    in_maps = []
    for core_jobs in jobs:
        lhsT = np.zeros((128, G, T, 128), np.float32)
        rhs = np.zeros((128, G, T, F), np.float32)
        for g, terms in enumerate(core_jobs):
            for t, (lb, rb) in enumerate(terms):
                lhsT[:, g, t, :] = lb
                rhs[:, g, t, :] = rb
        in_maps.append({"lhsT": lhsT, "rhs": rhs})
    return in_maps


def kernel(**inputs):
    return _run(inputs)[0]


def _get_programs():
    if "LA" not in _CACHE:
        _CACHE["LA"] = _build_banded()
        _CACHE["LB1"] = _build_combine(16, 4, 192, "LB1")
        _CACHE["LB2"] = _build_combine(8, 4, 256, "LB2")
        _CACHE["LC"] = _build_combine(4, 4, 384, "LC")
        _CACHE["LD"] = _build_combine(2, 6, 512, "LD")
        _CACHE["LE"] = _build_combine(2, 8, 256, "LE")
    return _CACHE


def _lhsT_blk(Mre, Mim, irows, kwin):
    """A[irows, kwin].T as (re, im) [128, 128] blocks."""
    br, bi = _pad_block(Mre, Mim, irows, kwin)
    return np.ascontiguousarray(br.T), np.ascontiguousarray(bi.T)


def _combine_jobs(Alist, Blist, ivals, kwin_of, cwin_of):
    """Per (A, B, i): groups (re, im), terms over kwins x {re-parts}.

    Alist/Blist: list of dense (re, im) pairs, one per combine.
    ivals: list of (combine_idx, i) assigned to this core, in group order.
    kwin_of(i) -> list of (k0, k1); cwin_of(i) -> (c0, c1).
    Returns list of groups for _pack_combine (2 groups per ival)."""
    groups = []
    for (ci, i) in ivals:
        Are, Aim = Alist[ci]
        Bre, Bim = Blist[ci]
        irows = (128 * i, 128 * (i + 1))
        c0, c1 = cwin_of(i)
        terms_re, terms_im = [], []
        for (k0, k1) in kwin_of(i):
            ArT, AiT = _lhsT_blk(Are, Aim, irows, (k0, k1))
            Br, Bi = _pad_block(Bre, Bim, (k0, k1), (c0, c1))
            terms_re.append((ArT, Br))
            terms_re.append((-AiT, Bi))
            terms_im.append((AiT, Br))
            terms_im.append((ArT, Bi))
        groups.append(terms_re)
        groups.append(terms_im)
    return groups


def _scatter(out, groups_data, ivals, cwin_of, rows_base=0):
    """Accumulate combine outputs back into dense (re, im) arrays."""
    for gi, (ci, i) in enumerate(ivals):
        c0, c1 = cwin_of(i)
        cc0, cc1 = max(c0, 0), min(c1, N)
        re = groups_data[2 * gi]
        im = groups_data[2 * gi + 1]
        Mre, Mim = out[ci]
        Mre[128 * i:128 * (i + 1), cc0:cc1] = re[:, cc0 - c0:cc1 - c0]
        Mim[128 * i:128 * (i + 1), cc0:cc1] = im[:, cc0 - c0:cc1 - c0]


def _exec(nc, in_maps, trace):
    try:
        res = run_bass_kernel_spmd(nc, in_maps, list(range(NCORES)), trace=trace)
    except Exception:
        # transient NRT exec hiccups resolve on retry
        import time
        time.sleep(15)
        res = run_bass_kernel_spmd(nc, in_maps, list(range(NCORES)), trace=trace)
    t = res.exec_time_ns or 0
    return res.results, t


def _run(inputs, trace=False, dev_probe=None):
    progs = _get_programs()
    inputs = {k: np.asarray(v) for k, v in inputs.items()}
    total_ns = 0

    # ---- LA: band build ----
    res, t = _exec(progs["LA"], _host_inputs_banded(inputs), trace)
    total_ns += t
    leaves = []  # dense (re, im) per leaf, order g = 4*d + q
    for d in range(NCORES):
        for q in range(NLEAF_C):
            M = _densify(res[d]["outE"][:, q], res[d]["outO"][:, q])
            leaves.append((np.ascontiguousarray(M.real),
                           np.ascontiguousarray(M.imag)))
    if dev_probe == "leaves":
        return leaves, total_ns

    # ---- LB1: CLo = leaf1@leaf0, CHiT = leaf2^T @ leaf3^T per core ----
    # groups per core: 8 (CLo: i x re/im) + 8 (CHiT: c x re/im)
    jobs = []
    for d in range(NCORES):
        l0, l1, l2, l3 = leaves[4 * d:4 * d + 4]
        groups = []
        # CLo = l1 @ l0: band +-16 each
        groups += _combine_jobs(
            [(l1[0], l1[1])], [(l0[0], l0[1])],
            [(0, i) for i in range(4)],
            lambda i: [(128 * i - 16, 128 * i + 112), (128 * i + 112, 128 * i + 240)],
            lambda i: (128 * i - 32, 128 * i + 160))
        # CHiT = l2^T @ l3^T: use transposed denses
        l2T = (np.ascontiguousarray(l2[0].T), np.ascontiguousarray(l2[1].T))
        l3T = (np.ascontiguousarray(l3[0].T), np.ascontiguousarray(l3[1].T))
        groups += _combine_jobs(
            [l2T], [l3T],
            [(0, i) for i in range(4)],
            lambda i: [(128 * i - 16, 128 * i + 112), (128 * i + 112, 128 * i + 240)],
            lambda i: (128 * i - 32, 128 * i + 160))
        jobs.append(groups)
    res, t = _exec(progs["LB1"], _pack_combine(jobs, 16, 4, 192), trace)
    total_ns += t

    z = lambda: (np.zeros((N, N), np.float32), np.zeros((N, N), np.float32))
    CLo = [z() for _ in range(NCORES)]
    CHiT = [z() for _ in range(NCORES)]
    for d in range(NCORES):
        o = res[d]["outv"]  # [128, 16, 192]
        gd = [o[:, g, :] for g in range(16)]
        _scatter(CLo[d:d + 1] * 1, gd[:8], [(0, i) for i in range(4)],
                 lambda i: (128 * i - 32, 128 * i + 160))
        _scatter([CHiT[d]], gd[8:], [(0, i) for i in range(4)],
                 lambda i: (128 * i - 32, 128 * i + 160))
    if dev_probe == "lb1":
        return (CLo, CHiT), total_ns

    # ---- LB2: P_d = CHi @ CLo  (lhsT = CHiT directly) ----
    jobs = []
    for d in range(NCORES):
        hT = CHiT[d]
        lo_ = CLo[d]
        groups = []
        for i in range(4):
            irows = (128 * i, 128 * (i + 1))
            c0, c1 = 128 * i - 64, 128 * i + 192
            terms_re, terms_im = [], []
            for (k0, k1) in ((128 * i - 32, 128 * i + 96),
                             (128 * i + 96, 128 * i + 224)):
                # lhsT block = CHi[irows, kwin].T = CHiT[kwin, irows]
                ArT, AiT = _pad_block(hT[0], hT[1], (k0, k1), irows)
                Br, Bi = _pad_block(lo_[0], lo_[1], (k0, k1), (c0, c1))
                terms_re.append((ArT, Br))
                terms_re.append((-AiT, Bi))
                terms_im.append((AiT, Br))
                terms_im.append((ArT, Bi))
            groups.append(terms_re)
            groups.append(terms_im)
        jobs.append(groups)
    res, t = _exec(progs["LB2"], _pack_combine(jobs, 8, 4, 256), trace)
    total_ns += t

    P = [z() for _ in range(NCORES)]
    for d in range(NCORES):
        o = res[d]["outv"]
        _scatter([P[d]], [o[:, g, :] for g in range(8)],
                 [(0, i) for i in range(4)],
                 lambda i: (128 * i - 64, 128 * i + 192))
    if dev_probe == "lb2":
        return P, total_ns

    # ---- LC: Q_j = P_{2j+1} @ P_{2j}, 2 cores per combine ----
    jobs = []
    for d in range(NCORES):
        j = d // 2
        iv = [(j, 2 * (d % 2)), (j, 2 * (d % 2) + 1)]
        groups = _combine_jobs(
            [(P[2 * jj + 1][0], P[2 * jj + 1][1]) for jj in range(4)],
            [(P[2 * jj][0], P[2 * jj][1]) for jj in range(4)],
            iv,
            lambda i: [(128 * i - 64, 128 * i + 64), (128 * i + 64, 128 * i + 192)],
            lambda i: (128 * i - 128, 128 * i + 256))
        jobs.append(groups)
    res, t = _exec(progs["LC"], _pack_combine(jobs, 4, 4, 384), trace)
    total_ns += t

    Q = [z() for _ in range(4)]
    for d in range(NCORES):
        j = d // 2
        iv = [(j, 2 * (d % 2)), (j, 2 * (d % 2) + 1)]
        _scatter(Q, [res[d]["outv"][:, g, :] for g in range(4)], iv,
                 lambda i: (128 * i - 128, 128 * i + 256))
    if dev_probe == "lc":
        return Q, total_ns

    # ---- LD: R_j = Q_{2j+1} @ Q_{2j}, 4 cores per combine ----
    jobs = []
    for d in range(NCORES):
        j = d // 4
        iv = [(j, d % 4)]
        groups = _combine_jobs(
            [(Q[2 * jj + 1][0], Q[2 * jj + 1][1]) for jj in range(2)],
            [(Q[2 * jj][0], Q[2 * jj][1]) for jj in range(2)],
            iv,
            lambda i: [(128 * i - 128, 128 * i), (128 * i, 128 * i + 128),
                       (128 * i + 128, 128 * i + 256)],
            lambda i: (0, N))
        jobs.append(groups)
    res, t = _exec(progs["LD"], _pack_combine(jobs, 2, 6, 512), trace)
    total_ns += t

    R = [z() for _ in range(2)]
    for d in range(NCORES):
        j = d // 4
        iv = [(j, d % 4)]
        _scatter(R, [res[d]["outv"][:, g, :] for g in range(2)], iv,
                 lambda i: (0, N))
    if dev_probe == "ld":
        return R, total_ns

    # ---- LE: M = R1 @ R0, core d -> (i = d//2, col half = d%2) ----
    jobs = []
    for d in range(NCORES):
        i, h = d // 2, d % 2
        irows = (128 * i, 128 * (i + 1))
        ccols = (256 * h, 256 * (h + 1))
        terms_re, terms_im = [], []
        for kk in range(4):
            kwin = (128 * kk, 128 * (kk + 1))
            ArT, AiT = _lhsT_blk(R[1][0], R[1][1], irows, kwin)
            Br, Bi = _pad_block(R[0][0], R[0][1], kwin, ccols)
            terms_re.append((ArT, Br))
            terms_re.append((-AiT, Bi))
            terms_im.append((AiT, Br))
            terms_im.append((ArT, Bi))
        jobs.append([terms_re, terms_im])
    res, t = _exec(progs["LE"], _pack_combine(jobs, 2, 8, 256), trace)
    total_ns += t

    M = np.zeros((N, N), np.complex64)
    for d in range(NCORES):
        i, h = d // 2, d % 2
        o = res[d]["outv"]
        M[128 * i:128 * (i + 1), 256 * h:256 * (h + 1)] = o[:, 0, :] + 1j * o[:, 1, :]
    return M, total_ns


# revision 3
# speedup vs baseline: 1.0778x; 1.0058x over previous
"""Two-phase Trainium2 kernel for the Clements mesh (N=512, 1024 layer apps).

Phase A (launch LA): each of the 8 cores builds 4 "leaf" products of 32
consecutive layer applications as banded 512x512 complex matrices (band
+-16) in E/O-parity band layout, using per-partition scalar mixes.
Phase B (launches LB1..LE): a binary matmul tree combines the 32 leaves
into the full transfer matrix on the TensorEngine; the host only
re-slices/transposes/zero-pads blocks between launches.
"""

import sys

sys.path.insert(0, "/opt/trn_rl_repo")

from contextlib import ExitStack

import numpy as np

import concourse.bass as bass
import concourse.tile as tile
from concourse import bacc, mybir
from concourse.bass_utils import run_bass_kernel_spmd

F32 = mybir.dt.float32
ALU = mybir.AluOpType
ACTF = mybir.ActivationFunctionType

N = 512
L = 512
TWO_PI = 6.283185307179586
HALF_PI = 1.5707963267948966
NCORES = 8

W = 40        # band tile width
A0 = 19       # diagonal position inside the band tile
NLEAF_C = 4   # leaves per core
STEPS = 8     # scan steps per leaf (32 layer applications)
WLO, WHI = 3, 36   # active band window (max half-width 16)

_CACHE = {}


# --------------------------------------------------------------------------
# Launch A: band build
# --------------------------------------------------------------------------

def _build_banded():
    nc = bacc.Bacc("TRN2", target_bir_lowering=False)

    par = {}
    # 64 even + 64 odd layer apps per core; columns ordered [step s: 16][leaf
    # q: 4][j: 2] so one dynamic slice per step grabs all 4 leaves' coeffs.
    for nm in ("the0", "the1", "le0", "le1", "ie0", "ie1",
               "tho0", "tho1", "lo0", "lo1", "io0", "io1"):
        par[nm] = nc.declare_dram_parameter(nm, [128, 64], F32, isOutput=False)
    par["pout"] = nc.declare_dram_parameter("pout", [128, 4], F32, isOutput=False)
    par["shifts"] = nc.declare_dram_parameter("shifts", [128, 5, 128], F32, isOutput=False)
    outE = nc.declare_dram_parameter("outE", [128, NLEAF_C, 2, 2, W], F32, isOutput=True)
    outO = nc.declare_dram_parameter("outO", [128, NLEAF_C, 2, 2, W], F32, isOutput=True)

    with tile.TileContext(nc) as tc, ExitStack() as ctx:
        consts = ctx.enter_context(tc.tile_pool(name="consts", bufs=1))
        coefp = ctx.enter_context(tc.tile_pool(name="coefs", bufs=1))
        srcp = ctx.enter_context(tc.tile_pool(name="srcs", bufs=1))
        stp = ctx.enter_context(tc.tile_pool(name="state", bufs=1))
        stgp = ctx.enter_context(tc.tile_pool(name="stage", bufs=2))
        psp = ctx.enter_context(tc.tile_pool(name="psum", bufs=1, space="PSUM"))

        shifts = consts.tile([128, 5, 128], F32, tag="shifts")
        nc.sync.dma_start(out=shifts[:], in_=par["shifts"][:])

        SINP = (-0.16666666639369604, 0.0083333316715976, -0.00019840942043806986,
                2.752917460996653e-06, -2.3955613511594512e-08)
        COSP = (-0.49999999647064386, 0.041666645176626854, -0.0013888464831511677,
                2.4765157753536994e-05, -2.6136488530828197e-07)
        PI_HI = 3.1415927410125732
        PI_LO = -8.742278012618954e-08

        def cos_sin(c_out, s_out, th, tagp, v=None):
            shape = list(th.shape)
            t_ = lambda nm: srcp.tile(shape, F32, tag=f"{tagp}{nm}", name=f"{tagp}{nm}")
            z, nz, a, mm, m2, p, acc, msk, w_, sm = (
                t_(n) for n in ("z", "nz", "a", "mm", "m2", "p", "acc", "msk", "w", "sm"))
            v = v or nc.vector
            v.tensor_scalar(out=z[:], in0=th[:], scalar1=-PI_HI, scalar2=None, op0=ALU.add)
            v.tensor_scalar(out=nz[:], in0=z[:], scalar1=-1.0, scalar2=None, op0=ALU.mult)
            v.tensor_tensor(out=a[:], in0=nz[:], in1=z[:], op=ALU.max)
            v.tensor_scalar(out=mm[:], in0=a[:], scalar1=-1.0, scalar2=PI_HI,
                            op0=ALU.mult, op1=ALU.add)
            v.tensor_scalar(out=msk[:], in0=mm[:], scalar1=1.0, scalar2=None,
                            op0=ALU.bypass)
            v.tensor_tensor(out=mm[:], in0=mm[:], in1=a[:], op=ALU.min)
            v.tensor_tensor(out=msk[:], in0=a[:], in1=msk[:], op=ALU.is_gt)
            nc.scalar.sign(w_[:], z[:])
            v.tensor_scalar(out=acc[:], in0=msk[:], scalar1=2.0, scalar2=-1.0,
                            op0=ALU.mult, op1=ALU.add)
            v.tensor_mul(w_[:], w_[:], acc[:])
            v.tensor_add(w_[:], w_[:], msk[:])
            v.tensor_scalar(out=w_[:], in0=w_[:], scalar1=PI_LO, scalar2=None, op0=ALU.mult)
            v.tensor_mul(m2[:], mm[:], mm[:])
            v.tensor_scalar(out=p[:], in0=m2[:], scalar1=SINP[4], scalar2=SINP[3],
                            op0=ALU.mult, op1=ALU.add)
            for cf in (SINP[2], SINP[1], SINP[0]):
                v.tensor_mul(p[:], p[:], m2[:])
                v.tensor_scalar(out=p[:], in0=p[:], scalar1=cf, scalar2=None, op0=ALU.add)
            v.tensor_mul(acc[:], m2[:], mm[:])
            v.tensor_mul(p[:], p[:], acc[:])
            v.tensor_add(sm[:], p[:], mm[:])
            v.tensor_scalar(out=p[:], in0=m2[:], scalar1=COSP[4], scalar2=COSP[3],
                            op0=ALU.mult, op1=ALU.add)
            for cf in (COSP[2], COSP[1], COSP[0]):
                v.tensor_mul(p[:], p[:], m2[:])
                v.tensor_scalar(out=p[:], in0=p[:], scalar1=cf, scalar2=None, op0=ALU.add)
            v.tensor_mul(p[:], p[:], m2[:])
            v.tensor_scalar(out=p[:], in0=p[:], scalar1=1.0, scalar2=None, op0=ALU.add)
            v.tensor_mul(acc[:], w_[:], p[:])
            v.tensor_add(acc[:], acc[:], sm[:])
            v.tensor_mul(sm[:], w_[:], sm[:])
            v.tensor_sub(p[:], p[:], sm[:])
            nc.scalar.sign(s_out[:], nz[:])
            v.tensor_mul(s_out[:], s_out[:], acc[:])
            v.tensor_scalar(out=acc[:], in0=a[:], scalar1=-HALF_PI, scalar2=None,
                            op0=ALU.add)
            nc.scalar.sign(c_out[:], acc[:])
            v.tensor_mul(c_out[:], c_out[:], p[:])

        # ---- per-layer coefficients: groups 0/1 = even pairs lo/hi,
        #      2/3 = odd pairs lo/hi; packed idx 0:t 1:r 2:A 3:B 4:Bn 5:rn
        #      6:E 7:En 8:Dn
        CO = {}
        for g, (thn, lon, ion) in enumerate(
            (("the0", "le0", "ie0"), ("the1", "le1", "ie1"),
             ("tho0", "lo0", "io0"), ("tho1", "lo1", "io1"))
        ):
            th = srcp.tile([128, 64], F32, tag=f"th{g}")
            lo = srcp.tile([128, 64], F32, tag=f"lo{g}")
            io = srcp.tile([128, 64], F32, tag=f"io{g}")
            nc.sync.dma_start(out=th[:], in_=par[thn][:])
            nc.sync.dma_start(out=lo[:], in_=par[lon][:])
            nc.sync.dma_start(out=io[:], in_=par[ion][:])

            ve = nc.vector
            u_ = srcp.tile([128, 64], F32, tag=f"u{g % 2}_")
            vp = srcp.tile([128, 64], F32, tag=f"vp{g % 2}")
            vm = srcp.tile([128, 64], F32, tag=f"vm{g % 2}")
            pp = srcp.tile([128, 64], F32, tag=f"pp{g % 2}")
            pm = srcp.tile([128, 64], F32, tag=f"pm{g % 2}")

            # packed coefficients: 0:t 1:r 2:rn 3:tc 4:ts 5:tsn 6:rt 7:rtn
            cg = coefp.tile([128, 8, 64], F32, tag=f"cg{g}")
            t_ = cg[:, 0, :]; r_ = cg[:, 1, :]; rn = cg[:, 2, :]
            tc = cg[:, 3, :]; ts_ = cg[:, 4, :]; tsn = cg[:, 5, :]
            rt = cg[:, 6, :]; rtn = cg[:, 7, :]
            cc = srcp.tile([128, 64], F32, tag=f"cc{g % 2}")
            ss = srcp.tile([128, 64], F32, tag=f"ss{g % 2}")

            ve.tensor_scalar(out=th[:], in0=th[:], scalar1=0.0,
                             scalar2=TWO_PI, op0=ALU.max, op1=ALU.min)
            cos_sin(cc, ss, th, f"cs{g % 2}", v=ve)
            ve.tensor_scalar(out=u_[:], in0=lo[:], scalar1=-1.0,
                             scalar2=1.0, op0=ALU.mult, op1=ALU.add)
            ve.tensor_scalar(out=vp[:], in0=io[:], scalar1=0.5,
                             scalar2=None, op0=ALU.add)
            ve.tensor_scalar(out=vm[:], in0=io[:], scalar1=-1.0,
                             scalar2=0.5, op0=ALU.mult, op1=ALU.add)
            ve.tensor_mul(pp[:], u_[:], vp[:])
            ve.tensor_mul(pm[:], u_[:], vm[:])

            def sqrt_ref(dst, x, y0t, rec):
                nc.scalar.activation(y0t[:], x[:], ACTF.Sqrt)
                ve.tensor_scalar(out=rec[:], in0=y0t[:], scalar1=1e-30,
                                 scalar2=None, op0=ALU.max)
                nc.vector.reciprocal(rec[:], rec[:])
                ve.tensor_mul(rec[:], x[:], rec[:])
                ve.tensor_add(rec[:], rec[:], y0t[:])
                ve.tensor_scalar(out=dst, in0=rec[:], scalar1=0.5,
                                 scalar2=None, op0=ALU.mult)

            sq_y0 = srcp.tile([128, 64], F32, tag=f"sqy{g % 2}")
            sq_rc = srcp.tile([128, 64], F32, tag=f"sqr{g % 2}")
            sqrt_ref(t_, pp, sq_y0, sq_rc)
            sqrt_ref(r_, pm, sq_y0, sq_rc)
            ve.tensor_scalar(out=rn, in0=r_, scalar1=-1.0,
                             scalar2=None, op0=ALU.mult)
            ve.tensor_mul(tc, t_, cc[:])
            ve.tensor_mul(ts_, t_, ss[:])
            ve.tensor_scalar(out=tsn, in0=ts_, scalar1=-1.0,
                             scalar2=None, op0=ALU.mult)
            nc.vector.reciprocal(u_[:], t_)       # 1/t  (t >= 0.65 by input range)
            ve.tensor_mul(rt, r_, u_[:])
            ve.tensor_scalar(out=rtn, in0=rt, scalar1=-1.0,
                             scalar2=None, op0=ALU.mult)
            CO[g] = cg

        # ---- per-leaf state ----
        ST = []
        for q in range(NLEAF_C):
            tiles = {}
            for nm in ("Ea", "Oa", "Eb", "Ob", "EsA", "EsB", "tmp"):
                t = stp.tile([128, 2, 2, W], F32, tag=f"{nm}{q}")
                nc.vector.memset(t[:], 0.0)
                tiles[nm] = t
            nc.vector.memset(tiles["Ea"][:, :, 0, A0:A0 + 1], 1.0)
            nc.vector.memset(tiles["Oa"][:, :, 0, A0:A0 + 1], 1.0)
            ST.append(tiles)

        SU = shifts[:, 0, :]
        SB = shifts[:, 1, :]
        SD = shifts[:, 2, :]
        S00 = shifts[:, 3, :]
        SB127 = shifts[:, 4, :]

        CIDX = dict(t=0, r=1, rn=2, tc=3, ts=4, tsn=5, rt=6, rtn=7)

        def mix(dst, srcT, srcB, tmp, grp, ci):
            """One PC+MMI band layer.  tmp = t*phase*top (2 ACT + 2 DVE);
            top outs = tmp + i r*bot[d-1] (2 DVE, in1=tmp); bottom =
            t*bot (1 GpSimd) + i (r/t)*tmp[d+1] (2 DVE)."""
            C = {k: CO[grp][:, v, ci:ci + 1] for k, v in CIDX.items()}
            v, g_, a_ = nc.vector, nc.gpsimd, nc.scalar

            def stt(out, in0, coef, in1):
                v.scalar_tensor_tensor(out=out, in0=in0, scalar=coef,
                                       in1=in1, op0=ALU.mult, op1=ALU.add)

            a_.activation(tmp["xt"], srcT["xt"], ACTF.Copy, bias=0.0, scale=C["tc"])
            a_.activation(tmp["yt"], srcT["xt"], ACTF.Copy, bias=0.0, scale=C["ts"])
            stt(tmp["xt"], srcT["yt"], C["tsn"], tmp["xt"])
            stt(tmp["yt"], srcT["yt"], C["tc"], tmp["yt"])
            g_.tensor_scalar(out=dst["xyb"], in0=srcB["xyb"], scalar1=C["t"],
                             scalar2=None, op0=ALU.mult)
            stt(dst["xt"], srcB["yb_m"], C["rn"], tmp["xt"])
            stt(dst["yt"], srcB["xb_m"], C["r"], tmp["yt"])
            stt(dst["xb"], tmp["yt_p"], C["rtn"], dst["xb"])
            stt(dst["yb"], tmp["xt_p"], C["rt"], dst["yb"])

        def win(s):
            h = 2 * s + 2
            return slice(A0 - h, A0 + h + 1)

        def aps_top(T, t, w):
            return dict(xt=T[:, t, 0, w], yt=T[:, t, 1, w])

        def aps_tmp(T, t, w, wp):
            return dict(xt=T[:, t, 0, w], yt=T[:, t, 1, w], xyt=T[:, t, :, w],
                        xt_p=T[:, t, 0, wp], yt_p=T[:, t, 1, wp])

        def aps_bot(T, t, w, wm):
            return dict(xb=T[:, t, 0, w], yb=T[:, t, 1, w],
                        xb_m=T[:, t, 0, wm], yb_m=T[:, t, 1, wm],
                        xyb=T[:, t, :, w])

        def aps_dst(TT, TB, t, w):
            return dict(xt=TT[:, t, 0, w], yt=TT[:, t, 1, w], xyt=TT[:, t, :, w],
                        xb=TB[:, t, 0, w], yb=TB[:, t, 1, w], xyb=TB[:, t, :, w])

        EV_SRC = ((("Ea", "Oa"), ("Eb", "Ob")), (("Eb", "Ob"), ("Ea", "Oa")))
        OD_SRC = ((("Oa", "esh"), ("Ob", "EsB")), (("Ob", "EsB"), ("Oa", "EsA")))

        for s in range(STEPS):
            w = win(s)
            wm = slice(w.start - 1, w.stop - 1)
            wp = slice(w.start + 1, w.stop + 1)
            wx = slice(w.start - 1, w.stop + 1)
            ci0 = 8 * s

            # two even layers, interleaved across leaves
            for jj in (0, 1):
                for q in range(NLEAF_C):
                    (sE, sO), (dE, dO) = EV_SRC[jj]
                    S = ST[q]
                    for t in (0, 1):
                        mix(aps_dst(S[dE], S[dO], t, w),
                            aps_top(S[sE], t, w),
                            aps_bot(S[sO], t, w, wm),
                            aps_tmp(S["tmp"], t, w, wp), t, ci0 + 2 * q + jj)

            # Esh[k] = E[k+1] for all leaves
            for q in range(NLEAF_C):
                Ea = ST[q]["Ea"]
                psh = psp.tile([128, 2, 2, W], F32, tag=f"psh{q}")
                nc.tensor.matmul(out=psh[:, 1, :, :], lhsT=SU, rhs=Ea[:, 1, :, :],
                                 start=True, stop=True)
                nc.tensor.matmul(out=psh[:, 0, :, :], lhsT=SU, rhs=Ea[:, 0, :, :],
                                 start=True, stop=False)
                nc.tensor.matmul(out=psh[:, 0, :, :], lhsT=SB, rhs=Ea[:, 1, :, :],
                                 start=False, stop=True)
                esh = stgp.tile([128, 2, 2, W], F32, tag=f"esh{q}")
                nc.scalar.copy(out=esh[:, :, :, wx], in_=psh[:, :, :, wx])
                ST[q]["esh"] = esh

            # two odd layers
            for jj in (0, 1):
                for q in range(NLEAF_C):
                    (sO, sB), (dO, dEs) = OD_SRC[jj]
                    S = ST[q]
                    for t in (0, 1):
                        mix(aps_dst(S[dO], S[dEs], t, w),
                            aps_top(S[sO], t, w),
                            aps_bot(S[sB], t, w, wm),
                            aps_tmp(S["tmp"], t, w, wp), 2 + t, ci0 + 2 * q + jj)

            # unshift: E'[k+1] = Es[k]; E'[0] = old E[0]
            for q in range(NLEAF_C):
                Ea, EsA = ST[q]["Ea"], ST[q]["EsA"]
                peb = psp.tile([128, 2, 2, W], F32, tag=f"peb{q}")
                nc.tensor.matmul(out=peb[:, 0, :, :], lhsT=SD, rhs=EsA[:, 0, :, :],
                                 start=True, stop=False)
                nc.tensor.matmul(out=peb[:, 0, :, :], lhsT=S00, rhs=Ea[:, 0, :, :],
                                 start=False, stop=True)
                nc.tensor.matmul(out=peb[:, 1, :, :], lhsT=SD, rhs=EsA[:, 1, :, :],
                                 start=True, stop=False)
                nc.tensor.matmul(out=peb[:, 1, :, :], lhsT=SB127, rhs=EsA[:, 0, :, :],
                                 start=False, stop=True)
                nc.scalar.copy(out=Ea[:, :, :, wx], in_=peb[:, :, :, wx])

        # ---- output phase rotation on leaf 3 (identity unless core 7) ----
        po = consts.tile([128, 4], F32, tag="po")
        co = consts.tile([128, 4], F32, tag="co")
        so = consts.tile([128, 4], F32, tag="so")
        son = consts.tile([128, 4], F32, tag="son")
        nc.sync.dma_start(out=po[:], in_=par["pout"][:])
        nc.vector.tensor_scalar(out=po[:], in0=po[:], scalar1=0.0,
                                scalar2=TWO_PI, op0=ALU.max, op1=ALU.min)
        cos_sin(co, so, po, "csout")
        nc.vector.tensor_scalar(out=son[:], in0=so[:], scalar1=-1.0,
                                scalar2=None, op0=ALU.mult)

        lastE, lastO = ST[3]["Ea"], ST[3]["Oa"]
        fE = stp.tile([128, 2, 2, W], F32, tag="fE")
        fO = stp.tile([128, 2, 2, W], F32, tag="fO")
        nc.vector.memset(fE[:], 0.0)
        nc.vector.memset(fO[:], 0.0)
        for (Sx, D, c0) in ((lastE, fE, 0), (lastO, fO, 2)):
            for t in (0, 1):
                cs = co[:, c0 + t: c0 + t + 1]
                ss = so[:, c0 + t: c0 + t + 1]
                sn = son[:, c0 + t: c0 + t + 1]
                v = nc.vector
                v.tensor_scalar(out=D[:, t, 0, w], in0=Sx[:, t, 0, w],
                                scalar1=cs, scalar2=None, op0=ALU.mult)
                v.scalar_tensor_tensor(out=D[:, t, 0, w], in0=Sx[:, t, 1, w],
                                       scalar=sn, in1=D[:, t, 0, w],
                                       op0=ALU.mult, op1=ALU.add)
                v.tensor_scalar(out=D[:, t, 1, w], in0=Sx[:, t, 1, w],
                                scalar1=cs, scalar2=None, op0=ALU.mult)
                v.scalar_tensor_tensor(out=D[:, t, 1, w], in0=Sx[:, t, 0, w],
                                       scalar=ss, in1=D[:, t, 1, w],
                                       op0=ALU.mult, op1=ALU.add)

        for q in range(NLEAF_C):
            Esrc = fE if q == 3 else ST[q]["Ea"]
            Osrc = fO if q == 3 else ST[q]["Oa"]
            nc.sync.dma_start(out=outE[:, q, :, :, :], in_=Esrc[:])
            nc.sync.dma_start(out=outO[:, q, :, :, :], in_=Osrc[:])

    nc.finalize()
    return nc


def _host_inputs_banded(inp):
    """Per-core input maps for launch A."""
    f = np.float32

    def packed(arr, pad_val=None):
        # arr: [512, P] layer-major -> per-core [2, 128, 64] with columns
        # ordered [s:16][q:4][jj:2]; even/odd pair groups split by 128.
        a = arr.T.astype(f)                      # [P, 512]
        if a.shape[0] == 255:
            pad = np.zeros((1, 512), f)
            if pad_val is not None:
                pad[:] = pad_val
            a = np.concatenate([a, pad], axis=0)  # [256, 512]
        out = []
        for d in range(NCORES):
            cols = a[:, 64 * d:64 * (d + 1)]      # layer apps of core d
            # device column order [s:8][q:4][jj:2]; source col = 16q + 2s + jj
            idx = np.array([16 * q + 2 * s + jj
                            for s in range(8) for q in range(4)
                            for jj in range(2)], np.int64)
            out.append(cols[:, idx])
        return out  # list of [256, 64]

    the = packed(inp["pc_even_phases"])
    le = packed(inp["mmi_loss_even"])
    ie = packed(inp["mmi_imb_even"])
    tho = packed(inp["pc_odd_phases"])
    lo = packed(inp["mmi_loss_odd"])
    io = packed(inp["mmi_imb_odd"], pad_val=None)
    # odd pad pair 255 -> identity: t=1, r=0 needs imb=0.5, loss=0, th=0
    for d in range(NCORES):
        io[d][255, :] = 0.5

    shifts = np.zeros((128, 5, 128), f)
    for p in range(127):
        shifts[p + 1, 0, p] = 1.0     # SU: out[p] = in[p+1]
        shifts[p, 2, p + 1] = 1.0     # SD: out[p+1] = in[p]
    shifts[0, 1, 127] = 1.0           # SB: out[127](t0) = in[0](t1)
    shifts[0, 3, 0] = 1.0             # S00
    shifts[127, 4, 0] = 1.0           # SB127: out[0](t1) = in[127](t0)

    pout = np.zeros((128, 4), f)
    p = np.arange(128)
    pc = inp["pc_out_phases"].astype(f)
    pout[:, 0] = pc[2 * p]
    pout[:, 1] = pc[256 + 2 * p]
    pout[:, 2] = pc[2 * p + 1]
    pout[:, 3] = pc[257 + 2 * p]

    in_maps = []
    for d in range(NCORES):
        m = {
            "the0": the[d][:128], "the1": the[d][128:],
            "le0": le[d][:128], "le1": le[d][128:],
            "ie0": ie[d][:128], "ie1": ie[d][128:],
            "tho0": tho[d][:128], "tho1": tho[d][128:],
            "lo0": lo[d][:128], "lo1": lo[d][128:],
            "io0": io[d][:128], "io1": io[d][128:],
            "shifts": shifts,
            "pout": pout if d == 7 else np.zeros((128, 4), f),
        }
        in_maps.append(m)
    return in_maps


def _densify(E, O):
    """Band tiles [128,2,2,W] -> dense complex64 [512, 512]."""
    M = np.zeros((N, N), np.complex64)
    d = np.arange(W) - A0
    for t in (0, 1):
        p = np.arange(128)
        for (T, r) in ((E, 2 * (p + 128 * t)), (O, 2 * (p + 128 * t) + 1)):
            cols = r[:, None] + d[None, :]
            ok = (cols >= 0) & (cols < N)
            vals = T[:, t, 0, :] + 1j * T[:, t, 1, :]
            M[np.broadcast_to(r[:, None], cols.shape)[ok], cols[ok]] = vals[ok]
    return M


# --------------------------------------------------------------------------
# Generic block-matmul combine launch
# --------------------------------------------------------------------------

def _build_combine(G, T, F, name, mmdt=F32):
    """out[:, g, :] = sum_t lhsT[:, g, t, :].T @ rhs[:, g, t, :].

    Inputs land in two half-batched DMAs each (pipelines DMA vs matmul);
    mmdt=float32r runs the PE at 1 cycle/row when F >= 256."""
    nc = bacc.Bacc("TRN2", target_bir_lowering=False)
    lhsT = nc.declare_dram_parameter("lhsT", [128, G, T, 128], mmdt, isOutput=False)
    rhs = nc.declare_dram_parameter("rhs", [128, G, T, F], mmdt, isOutput=False)
    outv = nc.declare_dram_parameter("outv", [128, G, F], F32, isOutput=True)

    GH = max(1, G // 2)

    with tile.TileContext(nc) as tc, ExitStack() as ctx:
        lp = ctx.enter_context(tc.tile_pool(name="lhs", bufs=1))
        rp = ctx.enter_context(tc.tile_pool(name="rhs", bufs=1))
        op = ctx.enter_context(tc.tile_pool(name="out", bufs=1))
        pp = ctx.enter_context(tc.tile_pool(name="ps", bufs=4, space="PSUM"))

        lt = lp.tile([128, G, T, 128], mmdt, tag="lt")
        rt = rp.tile([128, G, T, F], mmdt, tag="rt")
        ot = op.tile([128, G, F], F32, tag="ot")
        halves = [(0, GH), (GH, G)] if G > 1 else [(0, G)]
        for (g0, g1) in halves:
            nc.sync.dma_start(out=lt[:, g0:g1], in_=lhsT[:, g0:g1])
            nc.sync.dma_start(out=rt[:, g0:g1], in_=rhs[:, g0:g1])
        for g in range(G):
            ps = pp.tile([128, F], F32, tag="ps")
            for t in range(T):
                nc.tensor.matmul(out=ps[:], lhsT=lt[:, g, t, :], rhs=rt[:, g, t, :],
                                 start=(t == 0), stop=(t == T - 1))
            nc.scalar.copy(out=ot[:, g, :], in_=ps[:])
        nc.sync.dma_start(out=outv[:], in_=ot[:])

    nc.finalize()
    return nc


def _pad_block(Mre, Mim, rows, cols):
    """Zero-padded block [len(rows), len(cols)] from a dense matrix pair."""
    r0, r1 = rows
    c0, c1 = cols
    out_r = np.zeros((r1 - r0, c1 - c0), np.float32)
    out_i = np.zeros((r1 - r0, c1 - c0), np.float32)
    rr0, rr1 = max(r0, 0), min(r1, N)
    cc0, cc1 = max(c0, 0), min(c1, N)
    if rr1 > rr0 and cc1 > cc0:
        out_r[rr0 - r0:rr1 - r0, cc0 - c0:cc1 - c0] = Mre[rr0:rr1, cc0:cc1]
        out_i[rr0 - r0:rr1 - r0, cc0 - c0:cc1 - c0] = Mim[rr0:rr1, cc0:cc1]
    return out_r, out_i


def _pack_combine(jobs, G, T, F):
    """jobs: per core, list of G groups; each group = list of T (lhsT_blk,
    rhs_blk) fp32 pairs (128x128, 128xF).  Returns in_maps."""
    in_maps = []
    for core_jobs in jobs:
        lhsT = np.zeros((128, G, T, 128), np.float32)
        rhs = np.zeros((128, G, T, F), np.float32)
        for g, terms in enumerate(core_jobs):
            for t, (lb, rb) in enumerate(terms):
                lhsT[:, g, t, :] = lb
                rhs[:, g, t, :] = rb
        in_maps.append({"lhsT": lhsT, "rhs": rhs})
    return in_maps


def kernel(**inputs):
    return _run(inputs)[0]


def _get_programs():
    if "LA" not in _CACHE:
        F32R = mybir.dt.float32r
        _CACHE["LA"] = _build_banded()
        _CACHE["LB1"] = _build_combine(16, 4, 192, "LB1")
        _CACHE["LB2"] = _build_combine(8, 4, 256, "LB2", mmdt=F32R)
        _CACHE["LC"] = _build_combine(4, 4, 384, "LC", mmdt=F32R)
        _CACHE["LD"] = _build_combine(2, 6, 512, "LD", mmdt=F32R)
        _CACHE["LE"] = _build_combine(2, 8, 256, "LE", mmdt=F32R)
    return _CACHE


def _lhsT_blk(Mre, Mim, irows, kwin):
    """A[irows, kwin].T as (re, im) [128, 128] blocks."""
    br, bi = _pad_block(Mre, Mim, irows, kwin)
    return np.ascontiguousarray(br.T), np.ascontiguousarray(bi.T)


def _combine_jobs(Alist, Blist, ivals, kwin_of, cwin_of):
    """Per (A, B, i): groups (re, im), terms over kwins x {re-parts}.

    Alist/Blist: list of dense (re, im) pairs, one per combine.
    ivals: list of (combine_idx, i) assigned to this core, in group order.
    kwin_of(i) -> list of (k0, k1); cwin_of(i) -> (c0, c1).
    Returns list of groups for _pack_combine (2 groups per ival)."""
    groups = []
    for (ci, i) in ivals:
        Are, Aim = Alist[ci]
        Bre, Bim = Blist[ci]
        irows = (128 * i, 128 * (i + 1))
        c0, c1 = cwin_of(i)
        terms_re, terms_im = [], []
        for (k0, k1) in kwin_of(i):
            ArT, AiT = _lhsT_blk(Are, Aim, irows, (k0, k1))
            Br, Bi = _pad_block(Bre, Bim, (k0, k1), (c0, c1))
            terms_re.append((ArT, Br))
            terms_re.append((-AiT, Bi))
            terms_im.append((AiT, Br))
            terms_im.append((ArT, Bi))
        groups.append(terms_re)
        groups.append(terms_im)
    return groups


def _scatter(out, groups_data, ivals, cwin_of, rows_base=0):
    """Accumulate combine outputs back into dense (re, im) arrays."""
    for gi, (ci, i) in enumerate(ivals):
        c0, c1 = cwin_of(i)
        cc0, cc1 = max(c0, 0), min(c1, N)
        re = groups_data[2 * gi]
        im = groups_data[2 * gi + 1]
        Mre, Mim = out[ci]
        Mre[128 * i:128 * (i + 1), cc0:cc1] = re[:, cc0 - c0:cc1 - c0]
        Mim[128 * i:128 * (i + 1), cc0:cc1] = im[:, cc0 - c0:cc1 - c0]


def _exec(nc, in_maps, trace):
    try:
        res = run_bass_kernel_spmd(nc, in_maps, list(range(NCORES)), trace=trace)
    except Exception:
        # transient NRT exec hiccups resolve on retry
        import time
        time.sleep(15)
        res = run_bass_kernel_spmd(nc, in_maps, list(range(NCORES)), trace=trace)
    t = res.exec_time_ns or 0
    return res.results, t


def _run(inputs, trace=False, dev_probe=None):
    progs = _get_programs()
    inputs = {k: np.asarray(v) for k, v in inputs.items()}
    total_ns = 0

    # ---- LA: band build ----
    res, t = _exec(progs["LA"], _host_inputs_banded(inputs), trace)
    total_ns += t
    leaves = []  # dense (re, im) per leaf, order g = 4*d + q
    for d in range(NCORES):
        for q in range(NLEAF_C):
            M = _densify(res[d]["outE"][:, q], res[d]["outO"][:, q])
            leaves.append((np.ascontiguousarray(M.real),
                           np.ascontiguousarray(M.imag)))
    if dev_probe == "leaves":
        return leaves, total_ns

    # ---- LB1: CLo = leaf1@leaf0, CHiT = leaf2^T @ leaf3^T per core ----
    # groups per core: 8 (CLo: i x re/im) + 8 (CHiT: c x re/im)
    jobs = []
    for d in range(NCORES):
        l0, l1, l2, l3 = leaves[4 * d:4 * d + 4]
        groups = []
        # CLo = l1 @ l0: band +-16 each
        groups += _combine_jobs(
            [(l1[0], l1[1])], [(l0[0], l0[1])],
            [(0, i) for i in range(4)],
            lambda i: [(128 * i - 16, 128 * i + 112), (128 * i + 112, 128 * i + 240)],
            lambda i: (128 * i - 32, 128 * i + 160))
        # CHiT = l2^T @ l3^T: use transposed denses
        l2T = (np.ascontiguousarray(l2[0].T), np.ascontiguousarray(l2[1].T))
        l3T = (np.ascontiguousarray(l3[0].T), np.ascontiguousarray(l3[1].T))
        groups += _combine_jobs(
            [l2T], [l3T],
            [(0, i) for i in range(4)],
            lambda i: [(128 * i - 16, 128 * i + 112), (128 * i + 112, 128 * i + 240)],
            lambda i: (128 * i - 32, 128 * i + 160))
        jobs.append(groups)
    res, t = _exec(progs["LB1"], _pack_combine(jobs, 16, 4, 192), trace)
    total_ns += t

    z = lambda: (np.zeros((N, N), np.float32), np.zeros((N, N), np.float32))
    CLo = [z() for _ in range(NCORES)]
    CHiT = [z() for _ in range(NCORES)]
    for d in range(NCORES):
        o = res[d]["outv"]  # [128, 16, 192]
        gd = [o[:, g, :] for g in range(16)]
        _scatter(CLo[d:d + 1] * 1, gd[:8], [(0, i) for i in range(4)],
                 lambda i: (128 * i - 32, 128 * i + 160))
        _scatter([CHiT[d]], gd[8:], [(0, i) for i in range(4)],
                 lambda i: (128 * i - 32, 128 * i + 160))
    if dev_probe == "lb1":
        return (CLo, CHiT), total_ns

    # ---- LB2: P_d = CHi @ CLo  (lhsT = CHiT directly) ----
    jobs = []
    for d in range(NCORES):
        hT = CHiT[d]
        lo_ = CLo[d]
        groups = []
        for i in range(4):
            irows = (128 * i, 128 * (i + 1))
            c0, c1 = 128 * i - 64, 128 * i + 192
            terms_re, terms_im = [], []
            for (k0, k1) in ((128 * i - 32, 128 * i + 96),
                             (128 * i + 96, 128 * i + 224)):
                # lhsT block = CHi[irows, kwin].T = CHiT[kwin, irows]
                ArT, AiT = _pad_block(hT[0], hT[1], (k0, k1), irows)
                Br, Bi = _pad_block(lo_[0], lo_[1], (k0, k1), (c0, c1))
                terms_re.append((ArT, Br))
                terms_re.append((-AiT, Bi))
                terms_im.append((AiT, Br))
                terms_im.append((ArT, Bi))
            groups.append(terms_re)
            groups.append(terms_im)
        jobs.append(groups)
    res, t = _exec(progs["LB2"], _pack_combine(jobs, 8, 4, 256), trace)
    total_ns += t

    P = [z() for _ in range(NCORES)]
    for d in range(NCORES):
        o = res[d]["outv"]
        _scatter([P[d]], [o[:, g, :] for g in range(8)],
                 [(0, i) for i in range(4)],
                 lambda i: (128 * i - 64, 128 * i + 192))
    if dev_probe == "lb2":
        return P, total_ns

    # ---- LC: Q_j = P_{2j+1} @ P_{2j}, 2 cores per combine ----
    jobs = []
    for d in range(NCORES):
        j = d // 2
        iv = [(j, 2 * (d % 2)), (j, 2 * (d % 2) + 1)]
        groups = _combine_jobs(
            [(P[2 * jj + 1][0], P[2 * jj + 1][1]) for jj in range(4)],
            [(P[2 * jj][0], P[2 * jj][1]) for jj in range(4)],
            iv,
            lambda i: [(128 * i - 64, 128 * i + 64), (128 * i + 64, 128 * i + 192)],
            lambda i: (128 * i - 128, 128 * i + 256))
        jobs.append(groups)
    res, t = _exec(progs["LC"], _pack_combine(jobs, 4, 4, 384), trace)
    total_ns += t

    Q = [z() for _ in range(4)]
    for d in range(NCORES):
        j = d // 2
        iv = [(j, 2 * (d % 2)), (j, 2 * (d % 2) + 1)]
        _scatter(Q, [res[d]["outv"][:, g, :] for g in range(4)], iv,
                 lambda i: (128 * i - 128, 128 * i + 256))
    if dev_probe == "lc":
        return Q, total_ns

    # ---- LD: R_j = Q_{2j+1} @ Q_{2j}, 4 cores per combine ----
    jobs = []
    for d in range(NCORES):
        j = d // 4
        iv = [(j, d % 4)]
        groups = _combine_jobs(
            [(Q[2 * jj + 1][0], Q[2 * jj + 1][1]) for jj in range(2)],
            [(Q[2 * jj][0], Q[2 * jj][1]) for jj in range(2)],
            iv,
            lambda i: [(128 * i - 128, 128 * i), (128 * i, 128 * i + 128),
                       (128 * i + 128, 128 * i + 256)],
            lambda i: (0, N))
        jobs.append(groups)
    res, t = _exec(progs["LD"], _pack_combine(jobs, 2, 6, 512), trace)
    total_ns += t

    R = [z() for _ in range(2)]
    for d in range(NCORES):
        j = d // 4
        iv = [(j, d % 4)]
        _scatter(R, [res[d]["outv"][:, g, :] for g in range(2)], iv,
                 lambda i: (0, N))
    if dev_probe == "ld":
        return R, total_ns

    # ---- LE: M = R1 @ R0, core d -> (i = d//2, col half = d%2) ----
    jobs = []
    for d in range(NCORES):
        i, h = d // 2, d % 2
        irows = (128 * i, 128 * (i + 1))
        ccols = (256 * h, 256 * (h + 1))
        terms_re, terms_im = [], []
        for kk in range(4):
            kwin = (128 * kk, 128 * (kk + 1))
            ArT, AiT = _lhsT_blk(R[1][0], R[1][1], irows, kwin)
            Br, Bi = _pad_block(R[0][0], R[0][1], kwin, ccols)
            terms_re.append((ArT, Br))
            terms_re.append((-AiT, Bi))
            terms_im.append((AiT, Br))
            terms_im.append((ArT, Bi))
        jobs.append([terms_re, terms_im])
    res, t = _exec(progs["LE"], _pack_combine(jobs, 2, 8, 256), trace)
    total_ns += t

    M = np.zeros((N, N), np.complex64)
    for d in range(NCORES):
        i, h = d // 2, d % 2
        o = res[d]["outv"]
        M[128 * i:128 * (i + 1), 256 * h:256 * (h + 1)] = o[:, 0, :] + 1j * o[:, 1, :]
    return M, total_ns
